# revision 11
# baseline (speedup 1.0000x reference)
"""Trainium2 Bass kernel for the real-space Ewald potential.

Computes  out = NORM/(4*pi) * sum_{i!=j} (q_i . q_j) * erf(|r_i-r_j|/sqrt(2)) / |r_i-r_j|

Strategy (8 NeuronCores, SPMD):
  - The N x N pair grid is split into 8x8 super-tiles of 512x512; core c
    processes row c of the grid (all 4096 columns), rotated so the diagonal
    super-tile is always the core-local tile 0 (identical program, per-core
    data).
  - d2_ij = s_i + s_j - 2 r_i.r_j is produced directly by one K=5 matmul
    on the tensor engine: lhsT rows [-2rx,-2ry,-2rz, 1, s_j] x rhs rows
    [rx, ry, rz, s_i, 1].
  - d = sqrt(d2 + eps) and erf(d/sqrt(2)) on the scalar (ACT) engine in two
    strictly separated phases (sqrt and erf live in different ACT table
    sets; interleaving would reload tables at ~2.7us each time).
  - F = erf(d/sqrt(2)) / d on the vector engine (single divide op); the
    diagonal is zeroed via a precomputed mask on the diagonal super-tile.
  - G[c_q, i] = sum_j q[j,c_q] F[j,i] comes from a second K=128 matmul,
    accumulated in PSUM (four super-tiles share a PSUM bank via the PE
    column-tile quadrants 0/32/64/96); the final contraction
    sum_i q[i,c_q] G[c_q,i] is one fused multiply+reduce DVE op per bank
    plus a tiny ones-vector matmul. Each core emits one scalar partial;
    the host sums the 8 partials and applies the constant scale.
"""

import os
import sys

import numpy as np

for _p in ("/opt/trn_rl_repo",):
    if os.path.isdir(_p) and _p not in sys.path:
        sys.path.insert(0, _p)

import concourse.bacc as bacc  # noqa: E402
import concourse.mybir as mybir  # noqa: E402
import concourse.tile as tile  # noqa: E402
from concourse.bass_utils import run_bass_kernel_spmd  # noqa: E402

N = 4096  # atoms
NQ = 8  # charge channels
NCORES = 8
CH = 512  # super-tile edge (i-chunk width / j-chunk height)
NT = 8  # super-tiles processed per core
NGB = (NT + 3) // 4  # G PSUM banks (4 super-tiles per bank)
BIAS = 2e-5  # sqrt(d2 + BIAS): guards fp32 cancellation (|err| <~ 4e-6)
INV_SQRT2 = 0.7071067811865476
TWOPI = 2.0 * np.pi
NORM_FACTOR = 90.0474

TRACE = bool(os.environ.get("BASS_EWALD_TRACE"))
USE_FAST_RECIP = os.environ.get("BASS_EWALD_FAST_RECIP", "1") == "1"
LAST_RESULTS = None  # BassKernelResults of the most recent run (for test.py)

_prog = None


def _build_program():
    f32 = mybir.dt.float32
    AF = mybir.ActivationFunctionType
    OP = mybir.AluOpType

    nc = bacc.Bacc("TRN2", target_bir_lowering=False, debug=False, num_devices=NCORES)
    at_d = nc.dram_tensor("AT", [5, NT * CH], f32, kind="ExternalInput")
    bt_d = nc.dram_tensor("BT", [5, NT * CH], f32, kind="ExternalInput")
    qw_d = nc.dram_tensor("QW", [128, NT * 32], f32, kind="ExternalInput")
    qf_d = nc.dram_tensor("QF", [128, NGB * CH], f32, kind="ExternalInput")
    mk_d = nc.dram_tensor("MASK", [128, 4 * CH], f32, kind="ExternalInput")
    out_d = nc.dram_tensor("OUT", [1, 1], f32, kind="ExternalOutput")

    with tile.TileContext(nc) as tc:
        with (
            tc.tile_pool(name="const", bufs=1) as cp,
            tc.tile_pool(name="work", bufs=3) as wp,
            tc.tile_pool(name="single", bufs=1) as sp,
            tc.tile_pool(name="pd", bufs=2, space="PSUM") as pd,
            tc.tile_pool(name="pg", bufs=1, space="PSUM") as pg,
        ):
            at = cp.tile([5, NT * CH], f32)
            nc.sync.dma_start(at[:], at_d[:])
            bt = cp.tile([5, NT * CH], f32)
            nc.sync.dma_start(bt[:], bt_d[:])
            qw = cp.tile([128, NT * 32], f32)
            nc.sync.dma_start(qw[:], qw_d[:])
            qf = cp.tile([128, NGB * CH], f32)
            nc.sync.dma_start(qf[:], qf_d[:])
            mk = cp.tile([128, 4 * CH], f32)
            nc.sync.dma_start(mk[:], mk_d[:])
            dall = cp.tile([128, NT * 2048], f32)
            rdall = cp.tile([128, NT * 2048], f32)
            ones = cp.tile([128, 1], f32)
            nc.vector.memset(ones[:], 1.0)
            bias_t = cp.tile([128, 1], f32)
            nc.vector.memset(bias_t[:], BIAS)
            g = pg.tile([128, NGB * CH], f32, tag="g")
            nc.vector.memset(g[:], 0.0)

            # Phase 1: d2 matmuls + sqrt (sqrt ACT table set).
            for t in range(NT):
                for h in (0, 1):
                    ps = pd.tile([128, 1024], f32, tag="d2")
                    for u in (0, 1):
                        jb = 2 * h + u
                        nc.tensor.matmul(
                            ps[:, u * CH : (u + 1) * CH],
                            bt[:, t * CH + jb * 128 : t * CH + (jb + 1) * 128],
                            at[:, t * CH : (t + 1) * CH],
                            start=True,
                            stop=True,
                        )
                    dsl = dall[:, (2 * t + h) * 1024 : (2 * t + h + 1) * 1024]
                    nc.scalar.activation(dsl, ps[:], AF.Sqrt, bias=bias_t[:])
                    if USE_FAST_RECIP:
                        nc.vector.reciprocal_approx_fast(
                            out=rdall[:, (2 * t + h) * 1024 : (2 * t + h + 1) * 1024],
                            in_=dsl,
                        )
                    else:
                        nc.vector.reciprocal(
                            rdall[:, (2 * t + h) * 1024 : (2 * t + h + 1) * 1024],
                            dsl,
                        )

            # Keep the two ACT table sets in disjoint program ranges.
            tc.strict_bb_all_engine_barrier()

            # Phase 2: erf + divide (+ diagonal mask) + G matmuls.
            for t in range(NT):
                k, m = divmod(t, 4)  # G bank, quadrant
                for h in (0, 1):
                    dsl = dall[:, (2 * t + h) * 1024 : (2 * t + h + 1) * 1024]
                    rdsl = rdall[:, (2 * t + h) * 1024 : (2 * t + h + 1) * 1024]
                    e = wp.tile([128, 1024], f32, tag="e")
                    nc.scalar.activation(e[:], dsl, AF.Erf, scale=INV_SQRT2)
                    f = wp.tile([128, 1024], f32, tag="f")
                    if t == 0:
                        em = sp.tile([128, 1024], f32, tag="em")
                        nc.vector.tensor_tensor(
                            em[:], e[:], mk[:, h * 1024 : (h + 1) * 1024], OP.mult
                        )
                        nc.vector.tensor_tensor(f[:], em[:], rdsl, OP.mult)
                    else:
                        nc.vector.tensor_tensor(f[:], e[:], rdsl, OP.mult)
                    for u in (0, 1):
                        jb = 2 * h + u
                        nc.tensor.matmul(
                            g[32 * m : 32 * m + 8, k * CH : (k + 1) * CH],
                            qw[:, t * 32 + jb * 8 : t * 32 + (jb + 1) * 8],
                            f[:, u * CH : (u + 1) * CH],
                            start=(jb == 0),
                            stop=(jb == 3),
                            tile_position=(0, 32 * m),
                        )

            # Finalize: per G bank, fused multiply+reduce; then sum partitions.
            acc = sp.tile([128, NGB], f32, tag="acc")
            for k in range(NGB):
                prod = sp.tile([128, CH], f32, tag=f"prod{k}")
                nc.vector.tensor_tensor(
                    prod[:],
                    g[:, k * CH : (k + 1) * CH],
                    qf[:, k * CH : (k + 1) * CH],
                    OP.mult,
                )
                nc.vector.reduce_sum(
                    acc[:, k : k + 1], prod[:], axis=mybir.AxisListType.X
                )
            accsum = sp.tile([128, 1], f32, tag="accsum")
            nc.vector.reduce_sum(accsum[:], acc[:], axis=mybir.AxisListType.X)
            tot = pg.tile([1, 1], f32, tag="tot")
            nc.tensor.matmul(tot[:], accsum[:], ones[:], start=True, stop=True)
            res = sp.tile([1, 1], f32, tag="res")
            nc.scalar.copy(res[:], tot[:])
            nc.sync.dma_start(out_d[:], res[:])

    nc.compile()
    return nc


def _get_program():
    global _prog
    if _prog is None:
        _prog = _build_program()
    return _prog


def _host_prep(q, r):
    q = np.ascontiguousarray(np.asarray(q, np.float32))
    r = np.ascontiguousarray(np.asarray(r, np.float32))
    r64 = r.astype(np.float64)
    s = (r64 * r64).sum(1).astype(np.float32)
    onesN = np.ones(N, np.float32)
    A = np.stack([r[:, 0], r[:, 1], r[:, 2], s, onesN])  # [5, N] rhs rows
    B = np.stack([-2 * r[:, 0], -2 * r[:, 1], -2 * r[:, 2], onesN, s])  # lhsT rows
    qT = np.ascontiguousarray(q.T)  # [NQ, N]

    mk = np.ones((128, 4 * CH), np.float32)
    p = np.arange(128)
    for jb in range(4):
        mk[p, jb * CH + jb * 128 + p] = 0.0

    in_maps = []
    for c in range(NCORES):
        tiles = [(c, (c + t) % 8, 1.0) for t in range(NT)]  # (j-chunk, i-chunk, w)
        AT = np.empty((5, NT * CH), np.float32)
        BT = np.empty((5, NT * CH), np.float32)
        QW = np.empty((128, NT * 32), np.float32)
        QF = np.zeros((128, NGB * CH), np.float32)
        for t, (a, b, w) in enumerate(tiles):
            k, m = divmod(t, 4)
            AT[:, t * CH : (t + 1) * CH] = A[:, b * CH : (b + 1) * CH]
            BT[:, t * CH : (t + 1) * CH] = B[:, a * CH : (a + 1) * CH]
            QF[32 * m : 32 * m + NQ, k * CH : (k + 1) * CH] = qT[
                :, b * CH : (b + 1) * CH
            ]
            qa = (w * q[a * CH : (a + 1) * CH, :]).astype(np.float32)  # [512, NQ]
            QW[:, t * 32 : (t + 1) * 32] = (
                qa.reshape(4, 128, NQ).transpose(1, 0, 2).reshape(128, 32)
            )
        in_maps.append({"AT": AT, "BT": BT, "QW": QW, "QF": QF, "MASK": mk})
    return in_maps


def kernel(q, r, cell):
    global LAST_RESULTS
    in_maps = _host_prep(q, r)
    nc = _get_program()
    res = run_bass_kernel_spmd(nc, in_maps, list(range(NCORES)), trace=TRACE)
    LAST_RESULTS = res
    S = sum(float(res.results[c]["OUT"][0, 0]) for c in range(NCORES))
    val = S / TWOPI / 2.0 * NORM_FACTOR
    return np.array([val], np.float32)


# revision 12
# speedup vs baseline: 1.4764x; 1.4764x over previous
"""Trainium2 Bass kernel for the real-space Ewald potential.

Computes  out = NORM/(4*pi) * sum_{i!=j} (q_i . q_j) * erf(|r_i-r_j|/sqrt(2)) / |r_i-r_j|

Strategy (8 NeuronCores, SPMD):
  - The N x N pair grid is split into 8x8 super-tiles of 512x512; core c
    processes row c of the grid, rotated so the diagonal super-tile is the
    core-local tile 0 (identical program, per-core data).
  - d2_ij = s_i + s_j - 2 r_i.r_j comes from ONE K=18 bf16 matmul: r and s
    are split hi/lo (hi/mid/lo for s) into bf16 on the host, so the PE runs
    at full bf16 rate while keeping |d2 err| < 2e-4 (an fp32 matmul runs
    2 passes and is ~2-4x slower).
  - d = sqrt(d2 + 5e-4) and erf(d/sqrt(2)) on the scalar (ACT) engine in
    two strictly separated phases (sqrt and erf live in different ACT
    table sets; interleaving would reload tables at ~2.7us each time).
    1/d via the single-instruction DVE reciprocal_approx_fast in phase 1.
  - F = erf(d/sqrt(2)) * (1/d) multiplies are split between the vector and
    GPSIMD engines (bf16 output); the diagonal is zeroed via a precomputed
    mask on the diagonal super-tile.
  - G[c,i] = sum_j q[j,c] F[j,i] is a K=128 bf16 matmul with q ALSO split
    hi/lo (lhsT [qh|ql], M=16) so q's bf16 rounding cancels; four
    super-tiles share a PSUM bank via PE column-tile quadrants 0/32/64/96.
    The final contraction sum_i q[i,c] G[c,i] is a multiply+reduce on the
    vector engine plus a ones-vector matmul. Each core emits one scalar
    partial; the host sums the 8 partials and applies the constant scale.
"""

import os
import sys

import ml_dtypes
import numpy as np

for _p in ("/opt/trn_rl_repo",):
    if os.path.isdir(_p) and _p not in sys.path:
        sys.path.insert(0, _p)

import concourse.bacc as bacc  # noqa: E402
import concourse.mybir as mybir  # noqa: E402
import concourse.tile as tile  # noqa: E402
from concourse.bass_utils import run_bass_kernel_spmd  # noqa: E402

N = 4096  # atoms
NQ = 8  # charge channels
NCORES = 8
CH = 512  # super-tile edge (i-chunk width / j-chunk height)
NT = 8  # super-tiles processed per core
NGB = (NT + 3) // 4  # G PSUM banks (4 super-tiles per bank)
BIAS = 5e-4  # sqrt(d2 + BIAS): guards bf16-split cancellation (|err| < 2e-4)
INV_SQRT2 = 0.7071067811865476
TWOPI = 2.0 * np.pi
NORM_FACTOR = 90.0474
BF16 = ml_dtypes.bfloat16

# Units (t, h) whose F-multiply runs on GPSIMD instead of DVE (t>0 only;
# the masked diagonal tile t=0 keeps its 2-op chain on DVE).
GPSIMD_T = {5, 6, 7}

TRACE = bool(os.environ.get("BASS_EWALD_TRACE"))
LAST_RESULTS = None  # BassKernelResults of the most recent run (for test.py)

_prog = None


def _build_program():
    f32 = mybir.dt.float32
    bf16 = mybir.dt.bfloat16
    AF = mybir.ActivationFunctionType
    OP = mybir.AluOpType

    nc = bacc.Bacc("TRN2", target_bir_lowering=False, debug=False, num_devices=NCORES)
    at_d = nc.dram_tensor("AT", [18, NT * CH], bf16, kind="ExternalInput")
    bt_d = nc.dram_tensor("BT", [18, NT * CH], bf16, kind="ExternalInput")
    qw_d = nc.dram_tensor("QW", [128, NT * 64], bf16, kind="ExternalInput")
    qf_d = nc.dram_tensor("QF", [128, NGB * CH], f32, kind="ExternalInput")
    mk_d = nc.dram_tensor("MASK", [128, 4 * CH], f32, kind="ExternalInput")
    out_d = nc.dram_tensor("OUT", [1, 1], f32, kind="ExternalOutput")

    with tile.TileContext(nc) as tc:
        with (
            tc.tile_pool(name="const", bufs=1) as cp,
            tc.tile_pool(name="work", bufs=3) as wp,
            tc.tile_pool(name="single", bufs=1) as sp,
            tc.tile_pool(name="pd", bufs=2, space="PSUM") as pd,
            tc.tile_pool(name="pg", bufs=1, space="PSUM") as pg,
        ):
            at = cp.tile([18, NT * CH], bf16)
            nc.sync.dma_start(at[:], at_d[:])
            bt = cp.tile([18, NT * CH], bf16)
            nc.sync.dma_start(bt[:], bt_d[:])
            qw = cp.tile([128, NT * 64], bf16)
            nc.sync.dma_start(qw[:], qw_d[:])
            qf = cp.tile([128, NGB * CH], f32)
            nc.sync.dma_start(qf[:], qf_d[:])
            mk = cp.tile([128, 4 * CH], f32)
            nc.sync.dma_start(mk[:], mk_d[:])
            dall = cp.tile([128, NT * 2048], f32)
            rdall = cp.tile([128, NT * 2048], f32)
            ones = cp.tile([128, 1], f32)
            nc.vector.memset(ones[:], 1.0)
            bias_t = cp.tile([128, 1], f32)
            nc.vector.memset(bias_t[:], BIAS)
            g = pg.tile([128, NGB * CH], f32, tag="g")
            nc.vector.memset(g[:], 0.0)

            # Phase 1: d2 matmuls + sqrt (sqrt ACT table set) + 1/d on DVE.
            for t in range(NT):
                for h in (0, 1):
                    ps = pd.tile([128, 1024], f32, tag="d2")
                    for u in (0, 1):
                        jb = 2 * h + u
                        nc.tensor.matmul(
                            ps[:, u * CH : (u + 1) * CH],
                            bt[:, t * CH + jb * 128 : t * CH + (jb + 1) * 128],
                            at[:, t * CH : (t + 1) * CH],
                            start=True,
                            stop=True,
                        )
                    dsl = dall[:, (2 * t + h) * 1024 : (2 * t + h + 1) * 1024]
                    nc.scalar.activation(dsl, ps[:], AF.Sqrt, bias=bias_t[:])
                    nc.vector.reciprocal_approx_fast(
                        out=rdall[:, (2 * t + h) * 1024 : (2 * t + h + 1) * 1024],
                        in_=dsl,
                    )

            # Keep the two ACT table sets in disjoint program ranges.
            tc.strict_bb_all_engine_barrier()

            # Phase 2: erf + F-multiply (+ diagonal mask) + G matmuls.
            for t in range(NT):
                k, m = divmod(t, 4)  # G bank, quadrant
                for h in (0, 1):
                    dsl = dall[:, (2 * t + h) * 1024 : (2 * t + h + 1) * 1024]
                    rdsl = rdall[:, (2 * t + h) * 1024 : (2 * t + h + 1) * 1024]
                    e = wp.tile([128, 1024], f32, tag="e")
                    nc.scalar.activation(e[:], dsl, AF.Erf, scale=INV_SQRT2)
                    f = wp.tile([128, 1024], bf16, tag="f")
                    if t == 0:
                        em = sp.tile([128, 1024], f32, tag="em")
                        nc.vector.tensor_tensor(
                            em[:], e[:], mk[:, h * 1024 : (h + 1) * 1024], OP.mult
                        )
                        nc.vector.tensor_tensor(f[:], em[:], rdsl, OP.mult)
                    elif t in GPSIMD_T:
                        nc.gpsimd.tensor_tensor(f[:], e[:], rdsl, OP.mult)
                    else:
                        nc.vector.tensor_tensor(f[:], e[:], rdsl, OP.mult)
                    for u in (0, 1):
                        jb = 2 * h + u
                        nc.tensor.matmul(
                            g[32 * m : 32 * m + 16, k * CH : (k + 1) * CH],
                            qw[:, t * 64 + jb * 16 : t * 64 + (jb + 1) * 16],
                            f[:, u * CH : (u + 1) * CH],
                            start=(jb == 0),
                            stop=(jb == 3),
                            tile_position=(0, 32 * m),
                        )

            # Finalize: per G bank, multiply+reduce; then sum partitions.
            acc = sp.tile([128, NGB], f32, tag="acc")
            for k in range(NGB):
                prod = sp.tile([128, CH], f32, tag=f"prod{k}")
                nc.vector.tensor_tensor(
                    prod[:],
                    g[:, k * CH : (k + 1) * CH],
                    qf[:, k * CH : (k + 1) * CH],
                    OP.mult,
                )
                nc.vector.reduce_sum(
                    acc[:, k : k + 1], prod[:], axis=mybir.AxisListType.X
                )
            accsum = sp.tile([128, 1], f32, tag="accsum")
            nc.vector.reduce_sum(accsum[:], acc[:], axis=mybir.AxisListType.X)
            tot = pg.tile([1, 1], f32, tag="tot")
            nc.tensor.matmul(tot[:], accsum[:], ones[:], start=True, stop=True)
            res = sp.tile([1, 1], f32, tag="res")
            nc.scalar.copy(res[:], tot[:])
            nc.sync.dma_start(out_d[:], res[:])

    nc.compile()
    return nc


def _get_program():
    global _prog
    if _prog is None:
        _prog = _build_program()
    return _prog


def _bf16_split(x32, parts):
    """Split fp32 array into `parts` bf16 arrays summing to x32 (greedy)."""
    out = []
    rem = x32.astype(np.float64)
    for _ in range(parts):
        p = rem.astype(np.float32).astype(BF16)
        out.append(p)
        rem = rem - p.astype(np.float64)
    return out


def _host_prep(q, r):
    q = np.ascontiguousarray(np.asarray(q, np.float32))
    r = np.ascontiguousarray(np.asarray(r, np.float32))
    r64 = r.astype(np.float64)
    s64 = (r64 * r64).sum(1)

    rh, rl = _bf16_split(r, 2)  # [N,3] bf16 each
    m2rh, m2rl = (-2.0 * rh.astype(np.float32)).astype(BF16), (
        -2.0 * rl.astype(np.float32)
    ).astype(BF16)
    sh, sm, sl = _bf16_split(s64, 3)  # [N] bf16 each
    onesN = np.ones(N, BF16)

    # rhs rows (i side) pair with lhsT rows (j side), K=18:
    #   -2rh_j*rh_i, -2rh_j*rl_i, -2rl_j*rh_i, -2rl_j*rl_i (12 rows),
    #   (sh+sm+sl)_j * 1 (3 rows), 1 * (sh+sm+sl)_i (3 rows)
    A18 = np.concatenate(
        [rh.T, rl.T, rh.T, rl.T, [onesN, onesN, onesN], [sh, sm, sl]]
    ).astype(BF16)  # [18, N]
    B18 = np.concatenate(
        [m2rh.T, m2rh.T, m2rl.T, m2rl.T, [sh, sm, sl], [onesN, onesN, onesN]]
    ).astype(BF16)  # [18, N]

    qT = np.ascontiguousarray(q.T)  # [NQ, N] f32

    mk = np.ones((128, 4 * CH), np.float32)
    p = np.arange(128)
    for jb in range(4):
        mk[p, jb * CH + jb * 128 + p] = 0.0

    in_maps = []
    for c in range(NCORES):
        tiles = [(c, (c + t) % 8, 1.0) for t in range(NT)]  # (j-chunk, i-chunk, w)
        AT = np.empty((18, NT * CH), BF16)
        BT = np.empty((18, NT * CH), BF16)
        QW = np.empty((128, NT * 64), BF16)
        QF = np.zeros((128, NGB * CH), np.float32)
        for t, (a, b, w) in enumerate(tiles):
            k, m = divmod(t, 4)
            AT[:, t * CH : (t + 1) * CH] = A18[:, b * CH : (b + 1) * CH]
            BT[:, t * CH : (t + 1) * CH] = B18[:, a * CH : (a + 1) * CH]
            # Finalize reads quadrant rows 32m + [0..16): both the qh and ql
            # halves of G contract against the same fp32 qT chunk.
            QF[32 * m : 32 * m + NQ, k * CH : (k + 1) * CH] = qT[
                :, b * CH : (b + 1) * CH
            ]
            QF[32 * m + NQ : 32 * m + 2 * NQ, k * CH : (k + 1) * CH] = qT[
                :, b * CH : (b + 1) * CH
            ]
            wq = (w * q[a * CH : (a + 1) * CH, :]).astype(np.float32)  # [512, NQ]
            wqh, wql = _bf16_split(wq, 2)
            blk = np.concatenate([wqh, wql], axis=1)  # [512, 16]
            QW[:, t * 64 : (t + 1) * 64] = (
                blk.reshape(4, 128, 2 * NQ).transpose(1, 0, 2).reshape(128, 64)
            )
        in_maps.append({"AT": AT, "BT": BT, "QW": QW, "QF": QF, "MASK": mk})
    return in_maps


def kernel(q, r, cell):
    global LAST_RESULTS
    in_maps = _host_prep(q, r)
    nc = _get_program()
    res = run_bass_kernel_spmd(nc, in_maps, list(range(NCORES)), trace=TRACE)
    LAST_RESULTS = res
    S = sum(float(res.results[c]["OUT"][0, 0]) for c in range(NCORES))
    val = S / TWOPI / 2.0 * NORM_FACTOR
    return np.array([val], np.float32)


# revision 13
# speedup vs baseline: 2.0869x; 1.4136x over previous
"""Trainium2 Bass kernel for the real-space Ewald potential.

Computes  out = NORM/(4*pi) * sum_{i!=j} (q_i . q_j) * erf(|r_i-r_j|/sqrt(2)) / |r_i-r_j|

Strategy (8 NeuronCores, SPMD):
  - The N x N pair grid is split into 8x8 super-tiles of 512x512; core c
    processes row c of the grid, rotated so the diagonal super-tile is the
    core-local tile 0 (identical program, per-core data).
  - d2_ij = s_i + s_j - 2 r_i.r_j comes from ONE K=18 bf16 matmul: r and s
    are split hi/lo (hi/mid/lo for s) into bf16 on the host, so the PE runs
    at full bf16 rate while keeping |d2 err| < 2e-4 (an fp32 matmul runs
    2 passes and is ~2-4x slower).
  - d = sqrt(d2 + 5e-4) and erf(d/sqrt(2)) on the scalar (ACT) engine in
    two strictly separated phases (sqrt and erf live in different ACT
    table sets; interleaving would reload tables at ~2.7us each time).
    1/d via the single-instruction DVE reciprocal_approx_fast in phase 1.
  - F = erf(d/sqrt(2)) * (1/d) multiplies are split between the vector and
    GPSIMD engines (bf16 output); the diagonal is zeroed via a precomputed
    mask on the diagonal super-tile.
  - G[c,i] = sum_j q[j,c] F[j,i] is a K=128 bf16 matmul with q ALSO split
    hi/lo (lhsT [qh|ql], M=16) so q's bf16 rounding cancels; four
    super-tiles share a PSUM bank via PE column-tile quadrants 0/32/64/96.
    The final contraction sum_i q[i,c] G[c,i] is a multiply+reduce on the
    vector engine plus a ones-vector matmul. Each core emits one scalar
    partial; the host sums the 8 partials and applies the constant scale.
"""

import os
import sys

import ml_dtypes
import numpy as np

for _p in ("/opt/trn_rl_repo",):
    if os.path.isdir(_p) and _p not in sys.path:
        sys.path.insert(0, _p)

import concourse.bacc as bacc  # noqa: E402
import concourse.mybir as mybir  # noqa: E402
import concourse.tile as tile  # noqa: E402
from concourse.bass_utils import run_bass_kernel_spmd  # noqa: E402

N = 4096  # atoms
NQ = 8  # charge channels
NCORES = 8
CH = 512  # super-tile edge (i-chunk width / j-chunk height)
NT = 5  # super-tiles per core (1 diagonal + up to 4 off-diagonal x2-weighted)
NGB = (NT + 3) // 4  # G PSUM banks (4 super-tiles per bank)
BIAS = 5e-4  # sqrt(d2 + BIAS): guards bf16-split cancellation (|err| < 2e-4)
INV_SQRT2 = 0.7071067811865476
TWOPI = 2.0 * np.pi
NORM_FACTOR = 90.0474
BF16 = ml_dtypes.bfloat16

# Units (t, h) whose F-multiply runs on GPSIMD instead of DVE (t>0 only;
# the masked diagonal tile t=0 keeps its 2-op chain on DVE).
GPSIMD_T = {4}

TRACE = bool(os.environ.get("BASS_EWALD_TRACE"))
LAST_RESULTS = None  # BassKernelResults of the most recent run (for test.py)

_prog = None


def _build_program():
    f32 = mybir.dt.float32
    bf16 = mybir.dt.bfloat16
    AF = mybir.ActivationFunctionType
    OP = mybir.AluOpType

    nc = bacc.Bacc("TRN2", target_bir_lowering=False, debug=False, num_devices=NCORES)
    at_d = nc.dram_tensor("AT", [18, NT * CH], bf16, kind="ExternalInput")
    bt_d = nc.dram_tensor("BT", [18, NT * CH], bf16, kind="ExternalInput")
    qw_d = nc.dram_tensor("QW", [128, NT * 64], bf16, kind="ExternalInput")
    qf_d = nc.dram_tensor("QF", [128, NGB * CH], f32, kind="ExternalInput")
    mk_d = nc.dram_tensor("MASK", [128, 4 * CH], f32, kind="ExternalInput")
    out_d = nc.dram_tensor("OUT", [1, 1], f32, kind="ExternalOutput")

    with tile.TileContext(nc) as tc:
        with (
            tc.tile_pool(name="const", bufs=1) as cp,
            tc.tile_pool(name="work", bufs=3) as wp,
            tc.tile_pool(name="single", bufs=1) as sp,
            tc.tile_pool(name="pd", bufs=2, space="PSUM") as pd,
            tc.tile_pool(name="pg", bufs=1, space="PSUM") as pg,
        ):
            at = cp.tile([18, NT * CH], bf16)
            nc.sync.dma_start(at[:], at_d[:])
            bt = cp.tile([18, NT * CH], bf16)
            nc.sync.dma_start(bt[:], bt_d[:])
            qw = cp.tile([128, NT * 64], bf16)
            nc.sync.dma_start(qw[:], qw_d[:])
            qf = cp.tile([128, NGB * CH], f32)
            nc.sync.dma_start(qf[:], qf_d[:])
            mk = cp.tile([128, 4 * CH], f32)
            nc.sync.dma_start(mk[:], mk_d[:])
            dall = cp.tile([128, NT * 2048], f32)
            rdall = cp.tile([128, NT * 2048], f32)
            ones = cp.tile([128, 1], f32)
            nc.vector.memset(ones[:], 1.0)
            bias_t = cp.tile([128, 1], f32)
            nc.vector.memset(bias_t[:], BIAS)
            g = pg.tile([128, NGB * CH], f32, tag="g")
            nc.vector.memset(g[:], 0.0)

            # Phase 1: d2 matmuls + sqrt (sqrt ACT table set) + 1/d on DVE.
            for t in range(NT):
                for h in (0, 1):
                    ps = pd.tile([128, 1024], f32, tag="d2")
                    for u in (0, 1):
                        jb = 2 * h + u
                        nc.tensor.matmul(
                            ps[:, u * CH : (u + 1) * CH],
                            bt[:, t * CH + jb * 128 : t * CH + (jb + 1) * 128],
                            at[:, t * CH : (t + 1) * CH],
                            start=True,
                            stop=True,
                        )
                    dsl = dall[:, (2 * t + h) * 1024 : (2 * t + h + 1) * 1024]
                    nc.scalar.activation(dsl, ps[:], AF.Sqrt, bias=bias_t[:])
                    nc.vector.reciprocal_approx_fast(
                        out=rdall[:, (2 * t + h) * 1024 : (2 * t + h + 1) * 1024],
                        in_=dsl,
                    )

            # Keep the two ACT table sets in disjoint program ranges.
            tc.no_sync_barrier()

            # Phase 2: erf + F-multiply (+ diagonal mask) + G matmuls.
            for t in range(NT):
                k, m = divmod(t, 4)  # G bank, quadrant
                et = wp.tile([128, 2048], f32, tag="e")
                nc.scalar.activation(
                    et[:], dall[:, t * 2048 : (t + 1) * 2048], AF.Erf, scale=INV_SQRT2
                )
                for h in (0, 1):
                    esl = et[:, h * 1024 : (h + 1) * 1024]
                    rdsl = rdall[:, (2 * t + h) * 1024 : (2 * t + h + 1) * 1024]
                    f = wp.tile([128, 1024], bf16, tag="f")
                    if t == 0:
                        em = sp.tile([128, 1024], f32, tag="em")
                        nc.vector.tensor_tensor(
                            em[:], esl, mk[:, h * 1024 : (h + 1) * 1024], OP.mult
                        )
                        nc.vector.tensor_tensor(f[:], em[:], rdsl, OP.mult)
                    elif t in GPSIMD_T:
                        nc.gpsimd.tensor_tensor(f[:], esl, rdsl, OP.mult)
                    else:
                        nc.vector.tensor_tensor(f[:], esl, rdsl, OP.mult)
                    for u in (0, 1):
                        jb = 2 * h + u
                        nc.tensor.matmul(
                            g[32 * m : 32 * m + 16, k * CH : (k + 1) * CH],
                            qw[:, t * 64 + jb * 16 : t * 64 + (jb + 1) * 16],
                            f[:, u * CH : (u + 1) * CH],
                            start=(jb == 0),
                            stop=(jb == 3),
                            tile_position=(0, 32 * m),
                        )

            # Finalize: per G bank, multiply+reduce; then sum partitions.
            acc = sp.tile([128, NGB], f32, tag="acc")
            for k in range(NGB):
                prod = sp.tile([128, CH], f32, tag=f"prod{k}")
                nc.vector.tensor_tensor(
                    prod[:],
                    g[:, k * CH : (k + 1) * CH],
                    qf[:, k * CH : (k + 1) * CH],
                    OP.mult,
                )
                nc.vector.reduce_sum(
                    acc[:, k : k + 1], prod[:], axis=mybir.AxisListType.X
                )
            accsum = sp.tile([128, 1], f32, tag="accsum")
            nc.vector.reduce_sum(accsum[:], acc[:], axis=mybir.AxisListType.X)
            tot = pg.tile([1, 1], f32, tag="tot")
            nc.tensor.matmul(tot[:], accsum[:], ones[:], start=True, stop=True)
            res = sp.tile([1, 1], f32, tag="res")
            nc.scalar.copy(res[:], tot[:])
            nc.sync.dma_start(out_d[:], res[:])

    nc.compile()
    return nc


def _get_program():
    global _prog
    if _prog is None:
        _prog = _build_program()
    return _prog


def _bf16_split(x32, parts):
    """Split fp32 array into `parts` bf16 arrays summing to x32 (greedy)."""
    out = []
    rem = x32.astype(np.float64)
    for _ in range(parts):
        p = rem.astype(np.float32).astype(BF16)
        out.append(p)
        rem = rem - p.astype(np.float64)
    return out


def _host_prep(q, r):
    q = np.ascontiguousarray(np.asarray(q, np.float32))
    r = np.ascontiguousarray(np.asarray(r, np.float32))
    r64 = r.astype(np.float64)
    s64 = (r64 * r64).sum(1)

    rh, rl = _bf16_split(r, 2)  # [N,3] bf16 each
    m2rh, m2rl = (-2.0 * rh.astype(np.float32)).astype(BF16), (
        -2.0 * rl.astype(np.float32)
    ).astype(BF16)
    sh, sm, sl = _bf16_split(s64, 3)  # [N] bf16 each
    onesN = np.ones(N, BF16)

    # rhs rows (i side) pair with lhsT rows (j side), K=18:
    #   -2rh_j*rh_i, -2rh_j*rl_i, -2rl_j*rh_i, -2rl_j*rl_i (12 rows),
    #   (sh+sm+sl)_j * 1 (3 rows), 1 * (sh+sm+sl)_i (3 rows)
    A18 = np.concatenate(
        [rh.T, rl.T, rh.T, rl.T, [onesN, onesN, onesN], [sh, sm, sl]]
    ).astype(BF16)  # [18, N]
    B18 = np.concatenate(
        [m2rh.T, m2rh.T, m2rl.T, m2rl.T, [sh, sm, sl], [onesN, onesN, onesN]]
    ).astype(BF16)  # [18, N]

    qT = np.ascontiguousarray(q.T)  # [NQ, N] f32

    mk = np.ones((128, 4 * CH), np.float32)
    p = np.arange(128)
    for jb in range(4):
        mk[p, jb * CH + jb * 128 + p] = 0.0

    # 36 super-tiles of the symmetric pair grid: 8 diagonal (w=1, core-local
    # tile 0, diag-masked) + 28 upper-triangle pairs (w=2), dealt round-robin;
    # cores with only 3 pairs get a zero-weight dummy tile.
    pairs = [(a, b) for a in range(8) for b in range(a + 1, 8)]
    assignments = [[(c, c, 1.0)] for c in range(NCORES)]
    for idx, (a, b) in enumerate(pairs):
        assignments[idx % NCORES].append((a, b, 2.0))
    for c in range(NCORES):
        while len(assignments[c]) < NT:
            assignments[c].append((c, c, 0.0))

    in_maps = []
    for c in range(NCORES):
        tiles = assignments[c]  # (j-chunk a, i-chunk b, weight)
        AT = np.empty((18, NT * CH), BF16)
        BT = np.empty((18, NT * CH), BF16)
        QW = np.empty((128, NT * 64), BF16)
        QF = np.zeros((128, NGB * CH), np.float32)
        for t, (a, b, w) in enumerate(tiles):
            k, m = divmod(t, 4)
            AT[:, t * CH : (t + 1) * CH] = A18[:, b * CH : (b + 1) * CH]
            BT[:, t * CH : (t + 1) * CH] = B18[:, a * CH : (a + 1) * CH]
            # Finalize reads quadrant rows 32m + [0..16): both the qh and ql
            # halves of G contract against the same fp32 qT chunk.
            QF[32 * m : 32 * m + NQ, k * CH : (k + 1) * CH] = qT[
                :, b * CH : (b + 1) * CH
            ]
            QF[32 * m + NQ : 32 * m + 2 * NQ, k * CH : (k + 1) * CH] = qT[
                :, b * CH : (b + 1) * CH
            ]
            wq = (w * q[a * CH : (a + 1) * CH, :]).astype(np.float32)  # [512, NQ]
            wqh, wql = _bf16_split(wq, 2)
            blk = np.concatenate([wqh, wql], axis=1)  # [512, 16]
            QW[:, t * 64 : (t + 1) * 64] = (
                blk.reshape(4, 128, 2 * NQ).transpose(1, 0, 2).reshape(128, 64)
            )
        in_maps.append({"AT": AT, "BT": BT, "QW": QW, "QF": QF, "MASK": mk})
    return in_maps


def kernel(q, r, cell):
    global LAST_RESULTS
    in_maps = _host_prep(q, r)
    nc = _get_program()
    res = run_bass_kernel_spmd(nc, in_maps, list(range(NCORES)), trace=TRACE)
    LAST_RESULTS = res
    S = sum(float(res.results[c]["OUT"][0, 0]) for c in range(NCORES))
    val = S / TWOPI / 2.0 * NORM_FACTOR
    return np.array([val], np.float32)


# revision 14
# speedup vs baseline: 2.1907x; 1.0497x over previous
"""Trainium2 Bass kernel for the real-space Ewald potential.

Computes  out = NORM/(4*pi) * sum_{i!=j} (q_i . q_j) * erf(|r_i-r_j|/sqrt(2)) / |r_i-r_j|

Strategy (8 NeuronCores, SPMD):
  - The N x N pair grid is split into 8x8 super-tiles of 512x512; core c
    processes row c of the grid, rotated so the diagonal super-tile is the
    core-local tile 0 (identical program, per-core data).
  - d2_ij = s_i + s_j - 2 r_i.r_j comes from ONE K=18 bf16 matmul: r and s
    are split hi/lo (hi/mid/lo for s) into bf16 on the host, so the PE runs
    at full bf16 rate while keeping |d2 err| < 2e-4 (an fp32 matmul runs
    2 passes and is ~2-4x slower).
  - d = sqrt(d2 + 5e-4) and erf(d/sqrt(2)) on the scalar (ACT) engine in
    two strictly separated phases (sqrt and erf live in different ACT
    table sets; interleaving would reload tables at ~2.7us each time).
    1/d via the single-instruction DVE reciprocal_approx_fast in phase 1.
  - F = erf(d/sqrt(2)) * (1/d) multiplies are split between the vector and
    GPSIMD engines (bf16 output); the diagonal is zeroed via a precomputed
    mask on the diagonal super-tile.
  - G[c,i] = sum_j q[j,c] F[j,i] is a K=128 bf16 matmul with q ALSO split
    hi/lo (lhsT [qh|ql], M=16) so q's bf16 rounding cancels; four
    super-tiles share a PSUM bank via PE column-tile quadrants 0/32/64/96.
    The final contraction sum_i q[i,c] G[c,i] is a multiply+reduce on the
    vector engine plus a ones-vector matmul. Each core emits one scalar
    partial; the host sums the 8 partials and applies the constant scale.
"""

import os
import sys

import ml_dtypes
import numpy as np

for _p in ("/opt/trn_rl_repo",):
    if os.path.isdir(_p) and _p not in sys.path:
        sys.path.insert(0, _p)

import concourse.bacc as bacc  # noqa: E402
import concourse.mybir as mybir  # noqa: E402
import concourse.tile as tile  # noqa: E402
from concourse.bass_utils import run_bass_kernel_spmd  # noqa: E402

N = 4096  # atoms
NQ = 8  # charge channels
NCORES = 8
CH = 512  # super-tile edge (i-chunk width / j-chunk height)
NT = 5  # super-tiles per core (1 diagonal + up to 4 off-diagonal x2-weighted)
NGB = (NT + 3) // 4  # G PSUM banks (4 super-tiles per bank)
BIAS = 5e-4  # sqrt(d2 + BIAS): guards bf16-split cancellation (|err| < 2e-4)
INV_SQRT2 = 0.7071067811865476
TWOPI = 2.0 * np.pi
NORM_FACTOR = 90.0474
BF16 = ml_dtypes.bfloat16

# Super-tiles whose F-multiplies run on GPSIMD instead of DVE (early tiles
# only, so the slower GPSIMD ops don't drag the kernel tail).
GPSIMD_T = {1}

TRACE = bool(os.environ.get("BASS_EWALD_TRACE"))
LAST_RESULTS = None  # BassKernelResults of the most recent run (for test.py)

_prog = None


def _finalize_bank(nc, sp, g, qf, acc, k):
    OP = mybir.AluOpType
    f32 = mybir.dt.float32
    prod = sp.tile([128, CH], f32, tag=f"prod{k}")
    nc.vector.tensor_tensor(
        prod[:], g[:, k * CH : (k + 1) * CH], qf[:, k * CH : (k + 1) * CH], OP.mult
    )
    nc.vector.reduce_sum(acc[:, k : k + 1], prod[:], axis=mybir.AxisListType.X)


def _build_program():
    f32 = mybir.dt.float32
    bf16 = mybir.dt.bfloat16
    AF = mybir.ActivationFunctionType
    OP = mybir.AluOpType

    nc = bacc.Bacc("TRN2", target_bir_lowering=False, debug=False, num_devices=NCORES)
    at_d = nc.dram_tensor("AT", [18, NT * CH], bf16, kind="ExternalInput")
    bt_d = nc.dram_tensor("BT", [18, NT * CH], bf16, kind="ExternalInput")
    qw_d = nc.dram_tensor("QW", [128, NT * 64], bf16, kind="ExternalInput")
    qf_d = nc.dram_tensor("QF", [128, NGB * CH], f32, kind="ExternalInput")
    out_d = nc.dram_tensor("OUT", [1, 1], f32, kind="ExternalOutput")

    with tile.TileContext(nc) as tc:
        with (
            tc.tile_pool(name="const", bufs=1) as cp,
            tc.tile_pool(name="work", bufs=3) as wp,
            tc.tile_pool(name="single", bufs=1) as sp,
            tc.tile_pool(name="pd", bufs=2, space="PSUM") as pd,
            tc.tile_pool(name="pg", bufs=1, space="PSUM") as pg,
        ):
            at = cp.tile([18, NT * CH], bf16)
            nc.sync.dma_start(at[:], at_d[:])
            bt = cp.tile([18, NT * CH], bf16)
            nc.sync.dma_start(bt[:], bt_d[:])
            qw = cp.tile([128, NT * 64], bf16)
            nc.sync.dma_start(qw[:], qw_d[:])
            qf = cp.tile([128, NGB * CH], f32)
            dall = cp.tile([128, NT * 2048], f32)
            rdall = cp.tile([128, NT * 2048], f32)
            ones = cp.tile([128, 1], f32)
            nc.vector.memset(ones[:], 1.0)
            bias_t = cp.tile([128, 1], f32)
            nc.vector.memset(bias_t[:], BIAS)
            g = pg.tile([128, NGB * CH], f32, tag="g")
            nc.vector.memset(g[:], 0.0)

            # Phase 1: d2 matmuls + sqrt (sqrt ACT table set) + 1/d on DVE.
            for t in range(NT):
                for h in (0, 1):
                    ps = pd.tile([128, 1024], f32, tag="d2")
                    for u in (0, 1):
                        jb = 2 * h + u
                        nc.tensor.matmul(
                            ps[:, u * CH : (u + 1) * CH],
                            bt[:, t * CH + jb * 128 : t * CH + (jb + 1) * 128],
                            at[:, t * CH : (t + 1) * CH],
                            start=True,
                            stop=True,
                        )
                    dsl = dall[:, (2 * t + h) * 1024 : (2 * t + h + 1) * 1024]
                    nc.scalar.activation(dsl, ps[:], AF.Sqrt, bias=bias_t[:])
                    nc.vector.reciprocal_approx_fast(
                        out=rdall[:, (2 * t + h) * 1024 : (2 * t + h + 1) * 1024],
                        in_=dsl,
                    )

            # qf is only needed by the finalize stage; issuing its DMA after
            # phase 1 keeps the head of the sync queue clear for AT/BT.
            nc.sync.dma_start(qf[:], qf_d[:])

            # Keep the two ACT table sets in disjoint program ranges.
            tc.no_sync_barrier()

            # Phase 2: erf + F-multiply (+ diagonal mask) + G matmuls.
            for t in range(NT):
                k, m = divmod(t, 4)  # G bank, quadrant
                et = wp.tile([128, 2048], f32, tag="e")
                nc.scalar.activation(
                    et[:], dall[:, t * 2048 : (t + 1) * 2048], AF.Erf, scale=INV_SQRT2
                )
                for h in (0, 1):
                    esl = et[:, h * 1024 : (h + 1) * 1024]
                    rdsl = rdall[:, (2 * t + h) * 1024 : (2 * t + h + 1) * 1024]
                    f = wp.tile([128, 1024], bf16, tag="f")
                    if t in GPSIMD_T:
                        nc.gpsimd.tensor_tensor(f[:], esl, rdsl, OP.mult)
                    else:
                        nc.vector.tensor_tensor(f[:], esl, rdsl, OP.mult)
                    for u in (0, 1):
                        jb = 2 * h + u
                        nc.tensor.matmul(
                            g[32 * m : 32 * m + 16, k * CH : (k + 1) * CH],
                            qw[:, t * 64 + jb * 16 : t * 64 + (jb + 1) * 16],
                            f[:, u * CH : (u + 1) * CH],
                            start=(jb == 0),
                            stop=(jb == 3),
                            tile_position=(0, 32 * m),
                        )

            # Finalize: per G bank, multiply+reduce; then sum partitions.
            acc = sp.tile([128, NGB], f32, tag="acc")
            for k in range(NGB):
                _finalize_bank(nc, sp, g, qf, acc, k)
            accsum = sp.tile([128, 1], f32, tag="accsum")
            nc.vector.reduce_sum(accsum[:], acc[:], axis=mybir.AxisListType.X)
            tot = pg.tile([1, 1], f32, tag="tot")
            nc.tensor.matmul(tot[:], accsum[:], ones[:], start=True, stop=True)
            res = sp.tile([1, 1], f32, tag="res")
            nc.scalar.copy(res[:], tot[:])
            nc.sync.dma_start(out_d[:], res[:])

    nc.compile()
    return nc


def _get_program():
    global _prog
    if _prog is None:
        _prog = _build_program()
    return _prog


def _bf16_split(x32, parts):
    """Split fp32 array into `parts` bf16 arrays summing to x32 (greedy)."""
    out = []
    rem = x32.astype(np.float64)
    for _ in range(parts):
        p = rem.astype(np.float32).astype(BF16)
        out.append(p)
        rem = rem - p.astype(np.float64)
    return out


def _host_prep(q, r):
    q = np.ascontiguousarray(np.asarray(q, np.float32))
    r = np.ascontiguousarray(np.asarray(r, np.float32))
    r64 = r.astype(np.float64)
    s64 = (r64 * r64).sum(1)

    rh, rl = _bf16_split(r, 2)  # [N,3] bf16 each
    m2rh, m2rl = (-2.0 * rh.astype(np.float32)).astype(BF16), (
        -2.0 * rl.astype(np.float32)
    ).astype(BF16)
    sh, sm, sl = _bf16_split(s64, 3)  # [N] bf16 each
    onesN = np.ones(N, BF16)

    # rhs rows (i side) pair with lhsT rows (j side), K=18:
    #   -2rh_j*rh_i, -2rh_j*rl_i, -2rl_j*rh_i, -2rl_j*rl_i (12 rows),
    #   (sh+sm+sl)_j * 1 (3 rows), 1 * (sh+sm+sl)_i (3 rows)
    A18 = np.concatenate(
        [rh.T, rl.T, rh.T, rl.T, [onesN, onesN, onesN], [sh, sm, sl]]
    ).astype(BF16)  # [18, N]
    B18 = np.concatenate(
        [m2rh.T, m2rh.T, m2rl.T, m2rl.T, [sh, sm, sl], [onesN, onesN, onesN]]
    ).astype(BF16)  # [18, N]

    qT = np.ascontiguousarray(q.T)  # [NQ, N] f32

    # 36 super-tiles of the symmetric pair grid: 8 diagonal (w=1, core-local
    # tile 0, diag-masked) + 28 upper-triangle pairs (w=2), dealt round-robin;
    # cores with only 3 pairs get a zero-weight dummy tile.
    pairs = [(a, b) for a in range(8) for b in range(a + 1, 8)]
    assignments = [[(c, c, 1.0)] for c in range(NCORES)]
    for idx, (a, b) in enumerate(pairs):
        assignments[idx % NCORES].append((a, b, 2.0))
    for c in range(NCORES):
        while len(assignments[c]) < NT:
            assignments[c].append((c, c, 0.0))

    in_maps = []
    for c in range(NCORES):
        tiles = assignments[c]  # (j-chunk a, i-chunk b, weight)
        AT = np.empty((18, NT * CH), BF16)
        BT = np.empty((18, NT * CH), BF16)
        QW = np.empty((128, NT * 64), BF16)
        QF = np.zeros((128, NGB * CH), np.float32)
        for t, (a, b, w) in enumerate(tiles):
            k, m = divmod(t, 4)
            AT[:, t * CH : (t + 1) * CH] = A18[:, b * CH : (b + 1) * CH]
            BT[:, t * CH : (t + 1) * CH] = B18[:, a * CH : (a + 1) * CH]
            # Finalize reads quadrant rows 32m + [0..16): both the qh and ql
            # halves of G contract against the same fp32 qT chunk.
            QF[32 * m : 32 * m + NQ, k * CH : (k + 1) * CH] = qT[
                :, b * CH : (b + 1) * CH
            ]
            QF[32 * m + NQ : 32 * m + 2 * NQ, k * CH : (k + 1) * CH] = qT[
                :, b * CH : (b + 1) * CH
            ]
            wq = (w * q[a * CH : (a + 1) * CH, :]).astype(np.float32)  # [512, NQ]
            wqh, wql = _bf16_split(wq, 2)
            blk = np.concatenate([wqh, wql], axis=1)  # [512, 16]
            QW[:, t * 64 : (t + 1) * 64] = (
                blk.reshape(4, 128, 2 * NQ).transpose(1, 0, 2).reshape(128, 64)
            )
        in_maps.append({"AT": AT, "BT": BT, "QW": QW, "QF": QF})
    return in_maps


def _diag_constant():
    """F value the device computes on the (unmasked) pair-grid diagonal.

    d2 on the diagonal is |err| < 2e-4, and F(x) = erf(sqrt((x+B)/2)) /
    sqrt(x+B) is flat there (variation < 5e-5 relative), so every diagonal
    element lands on the same bf16 value: bf16(F(0)). The bf16 bucket is
    0.4% wide -- vastly wider than the variation -- so this is exact."""
    from scipy.special import erf as _erf

    d = np.sqrt(BIAS)
    c = float(_erf(d / np.sqrt(2.0)) / d)
    return float(np.float32(c).astype(BF16))


def kernel(q, r, cell):
    global LAST_RESULTS
    in_maps = _host_prep(q, r)
    nc = _get_program()
    res = run_bass_kernel_spmd(nc, in_maps, list(range(NCORES)), trace=TRACE)
    LAST_RESULTS = res
    S = sum(float(res.results[c]["OUT"][0, 0]) for c in range(NCORES))
    S -= _diag_constant() * float((q.astype(np.float64) ** 2).sum())
    val = S / TWOPI / 2.0 * NORM_FACTOR
    return np.array([val], np.float32)


# revision 15
# speedup vs baseline: 2.3216x; 1.0598x over previous
"""Trainium2 Bass kernel for the real-space Ewald potential.

Computes  out = NORM/(4*pi) * sum_{i!=j} (q_i . q_j) * erf(|r_i-r_j|/sqrt(2)) / |r_i-r_j|

Strategy (8 NeuronCores, SPMD):
  - The N x N pair grid is split into 8x8 super-tiles of 512x512; core c
    processes row c of the grid, rotated so the diagonal super-tile is the
    core-local tile 0 (identical program, per-core data).
  - d2_ij = s_i + s_j - 2 r_i.r_j comes from ONE K=18 bf16 matmul: r and s
    are split hi/lo (hi/mid/lo for s) into bf16 on the host, so the PE runs
    at full bf16 rate while keeping |d2 err| < 2e-4 (an fp32 matmul runs
    2 passes and is ~2-4x slower).
  - d = sqrt(d2 + 5e-4) and erf(d/sqrt(2)) on the scalar (ACT) engine in
    two strictly separated phases (sqrt and erf live in different ACT
    table sets; interleaving would reload tables at ~2.7us each time).
    1/d via the single-instruction DVE reciprocal_approx_fast in phase 1.
  - F = erf(d/sqrt(2)) * (1/d) multiplies are split between the vector and
    GPSIMD engines (bf16 output); the diagonal is zeroed via a precomputed
    mask on the diagonal super-tile.
  - G[c,i] = sum_j q[j,c] F[j,i] is a K=128 bf16 matmul with q ALSO split
    hi/lo (lhsT [qh|ql], M=16) so q's bf16 rounding cancels; four
    super-tiles share a PSUM bank via PE column-tile quadrants 0/32/64/96.
    The final contraction sum_i q[i,c] G[c,i] is a multiply+reduce on the
    vector engine plus a ones-vector matmul. Each core emits one scalar
    partial; the host sums the 8 partials and applies the constant scale.
"""

import os
import sys

import ml_dtypes
import numpy as np

for _p in ("/opt/trn_rl_repo",):
    if os.path.isdir(_p) and _p not in sys.path:
        sys.path.insert(0, _p)

import concourse.bacc as bacc  # noqa: E402
import concourse.mybir as mybir  # noqa: E402
import concourse.tile as tile  # noqa: E402
from concourse.bass_utils import run_bass_kernel_spmd  # noqa: E402

N = 4096  # atoms
NQ = 8  # charge channels
NCORES = 8
CH = 512  # super-tile edge (i-chunk width / j-chunk height)
NT = 5  # super-tiles per core (1 diagonal + up to 4 off-diagonal x2-weighted)
NGB = (NT + 3) // 4  # G PSUM banks (4 super-tiles per bank)
BIAS = 5e-4  # sqrt(d2 + BIAS): guards bf16-split cancellation (|err| < 2e-4)
INV_SQRT2 = 0.7071067811865476
TWOPI = 2.0 * np.pi
NORM_FACTOR = 90.0474
BF16 = ml_dtypes.bfloat16

# GPSIMD offload disabled: concurrent GPSIMD+DVE ops contend on SBUF ports
# (measured: a 1.1us DVE multiply stretches to 3.2us next to a GPSIMD one).
GPSIMD_T = set()

TRACE = bool(os.environ.get("BASS_EWALD_TRACE"))
LAST_RESULTS = None  # BassKernelResults of the most recent run (for test.py)

_prog = None


def _finalize_bank(nc, sp, gk, qf, acc, k):
    OP = mybir.AluOpType
    f32 = mybir.dt.float32
    prod = sp.tile([128, CH], f32, tag=f"prod{k}")
    nc.vector.tensor_tensor(
        prod[:], gk[:], qf[:, k * CH : (k + 1) * CH], OP.mult
    )
    nc.vector.reduce_sum(acc[:, k : k + 1], prod[:], axis=mybir.AxisListType.X)


def _build_program():
    f32 = mybir.dt.float32
    bf16 = mybir.dt.bfloat16
    AF = mybir.ActivationFunctionType
    OP = mybir.AluOpType

    nc = bacc.Bacc("TRN2", target_bir_lowering=False, debug=False, num_devices=NCORES)
    at_d = nc.dram_tensor("AT", [18, NT * CH], bf16, kind="ExternalInput")
    bt_d = nc.dram_tensor("BT", [18, NT * CH], bf16, kind="ExternalInput")
    qw_d = nc.dram_tensor("QW", [128, NT * 64], bf16, kind="ExternalInput")
    qf_d = nc.dram_tensor("QF", [128, NGB * CH], f32, kind="ExternalInput")
    out_d = nc.dram_tensor("OUT", [1, 1], f32, kind="ExternalOutput")

    with tile.TileContext(nc) as tc:
        with (
            tc.tile_pool(name="const", bufs=1) as cp,
            tc.tile_pool(name="work", bufs=3) as wp,
            tc.tile_pool(name="single", bufs=1) as sp,
            tc.tile_pool(name="pd", bufs=2, space="PSUM") as pd,
            tc.tile_pool(name="pg", bufs=1, space="PSUM") as pg,
        ):
            at = cp.tile([18, NT * CH], bf16)
            nc.sync.dma_start(at[:], at_d[:])
            bt = cp.tile([18, NT * CH], bf16)
            nc.sync.dma_start(bt[:], bt_d[:])
            qw = cp.tile([128, NT * 64], bf16)
            nc.gpsimd.dma_start(qw[:], qw_d[:])
            qf = cp.tile([128, NGB * CH], f32)
            dall = cp.tile([128, NT * 2048], f32)
            rdall = cp.tile([128, NT * 2048], f32)
            ones = cp.tile([128, 1], f32)
            nc.vector.memset(ones[:], 1.0)
            bias_t = cp.tile([128, 1], f32)
            nc.vector.memset(bias_t[:], BIAS)
            gbanks = []
            for k in range(NGB):
                gk = pg.tile([128, CH], f32, tag=f"g{k}")
                nc.vector.memset(gk[:], 0.0)
                gbanks.append(gk)

            # Phase 1: d2 matmuls + sqrt (sqrt ACT table set) + 1/d on DVE.
            for t in range(NT):
                for h in (0, 1):
                    ps = pd.tile([128, 1024], f32, tag="d2")
                    for u in (0, 1):
                        jb = 2 * h + u
                        nc.tensor.matmul(
                            ps[:, u * CH : (u + 1) * CH],
                            bt[:, t * CH + jb * 128 : t * CH + (jb + 1) * 128],
                            at[:, t * CH : (t + 1) * CH],
                            start=True,
                            stop=True,
                        )
                    dsl = dall[:, (2 * t + h) * 1024 : (2 * t + h + 1) * 1024]
                    nc.scalar.activation(dsl, ps[:], AF.Sqrt, bias=bias_t[:])
                    nc.vector.reciprocal_approx_fast(
                        out=rdall[:, (2 * t + h) * 1024 : (2 * t + h + 1) * 1024],
                        in_=dsl,
                    )

            # qf is only needed by the finalize stage; issuing its DMA after
            # phase 1 keeps the head of the sync queue clear for AT/BT.
            nc.gpsimd.dma_start(qf[:], qf_d[:])

            # Keep the two ACT table sets in disjoint program ranges.
            tc.no_sync_barrier()

            # Phase 2: erf + F-multiply (+ diagonal mask) + G matmuls.
            acc = sp.tile([128, NGB], f32, tag="acc")
            for t in range(NT):
                k, m = divmod(t, 4)  # G bank, quadrant
                et = wp.tile([128, 2048], f32, tag="e")
                nc.scalar.activation(
                    et[:], dall[:, t * 2048 : (t + 1) * 2048], AF.Erf, scale=INV_SQRT2
                )
                for h in (0, 1):
                    esl = et[:, h * 1024 : (h + 1) * 1024]
                    rdsl = rdall[:, (2 * t + h) * 1024 : (2 * t + h + 1) * 1024]
                    f = wp.tile([128, 1024], bf16, tag="f")
                    if t in GPSIMD_T:
                        nc.gpsimd.tensor_tensor(f[:], esl, rdsl, OP.mult)
                    else:
                        nc.vector.tensor_tensor(f[:], esl, rdsl, OP.mult)
                    for u in (0, 1):
                        jb = 2 * h + u
                        nc.tensor.matmul(
                            gbanks[k][32 * m : 32 * m + 16, :],
                            qw[:, t * 64 + jb * 16 : t * 64 + (jb + 1) * 16],
                            f[:, u * CH : (u + 1) * CH],
                            start=(jb == 0),
                            stop=(jb == 3),
                            tile_position=(0, 32 * m),
                        )

            # Finalize: per G bank, multiply+reduce; then sum partitions.
            for k in range(NGB):
                _finalize_bank(nc, sp, gbanks[k], qf, acc, k)
            accsum = sp.tile([128, 1], f32, tag="accsum")
            nc.vector.reduce_sum(accsum[:], acc[:], axis=mybir.AxisListType.X)
            tot = pg.tile([1, 1], f32, tag="tot")
            nc.tensor.matmul(tot[:], accsum[:], ones[:], start=True, stop=True)
            res = sp.tile([1, 1], f32, tag="res")
            nc.scalar.copy(res[:], tot[:])
            nc.sync.dma_start(out_d[:], res[:])

    nc.compile()
    return nc


def _get_program():
    global _prog
    if _prog is None:
        _prog = _build_program()
    return _prog


def _bf16_split(x32, parts):
    """Split fp32 array into `parts` bf16 arrays summing to x32 (greedy)."""
    out = []
    rem = x32.astype(np.float64)
    for _ in range(parts):
        p = rem.astype(np.float32).astype(BF16)
        out.append(p)
        rem = rem - p.astype(np.float64)
    return out


def _host_prep(q, r):
    q = np.ascontiguousarray(np.asarray(q, np.float32))
    r = np.ascontiguousarray(np.asarray(r, np.float32))
    r64 = r.astype(np.float64)
    s64 = (r64 * r64).sum(1)

    rh, rl = _bf16_split(r, 2)  # [N,3] bf16 each
    m2rh, m2rl = (-2.0 * rh.astype(np.float32)).astype(BF16), (
        -2.0 * rl.astype(np.float32)
    ).astype(BF16)
    sh, sm, sl = _bf16_split(s64, 3)  # [N] bf16 each
    onesN = np.ones(N, BF16)

    # rhs rows (i side) pair with lhsT rows (j side), K=18:
    #   -2rh_j*rh_i, -2rh_j*rl_i, -2rl_j*rh_i, -2rl_j*rl_i (12 rows),
    #   (sh+sm+sl)_j * 1 (3 rows), 1 * (sh+sm+sl)_i (3 rows)
    A18 = np.concatenate(
        [rh.T, rl.T, rh.T, rl.T, [onesN, onesN, onesN], [sh, sm, sl]]
    ).astype(BF16)  # [18, N]
    B18 = np.concatenate(
        [m2rh.T, m2rh.T, m2rl.T, m2rl.T, [sh, sm, sl], [onesN, onesN, onesN]]
    ).astype(BF16)  # [18, N]

    qT = np.ascontiguousarray(q.T)  # [NQ, N] f32

    # 36 super-tiles of the symmetric pair grid: 8 diagonal (w=1, core-local
    # tile 0, diag-masked) + 28 upper-triangle pairs (w=2), dealt round-robin;
    # cores with only 3 pairs get a zero-weight dummy tile.
    pairs = [(a, b) for a in range(8) for b in range(a + 1, 8)]
    assignments = [[(c, c, 1.0)] for c in range(NCORES)]
    for idx, (a, b) in enumerate(pairs):
        assignments[idx % NCORES].append((a, b, 2.0))
    for c in range(NCORES):
        while len(assignments[c]) < NT:
            assignments[c].append((c, c, 0.0))

    in_maps = []
    for c in range(NCORES):
        tiles = assignments[c]  # (j-chunk a, i-chunk b, weight)
        AT = np.empty((18, NT * CH), BF16)
        BT = np.empty((18, NT * CH), BF16)
        QW = np.empty((128, NT * 64), BF16)
        QF = np.zeros((128, NGB * CH), np.float32)
        for t, (a, b, w) in enumerate(tiles):
            k, m = divmod(t, 4)
            AT[:, t * CH : (t + 1) * CH] = A18[:, b * CH : (b + 1) * CH]
            BT[:, t * CH : (t + 1) * CH] = B18[:, a * CH : (a + 1) * CH]
            # Finalize reads quadrant rows 32m + [0..16): both the qh and ql
            # halves of G contract against the same fp32 qT chunk.
            QF[32 * m : 32 * m + NQ, k * CH : (k + 1) * CH] = qT[
                :, b * CH : (b + 1) * CH
            ]
            QF[32 * m + NQ : 32 * m + 2 * NQ, k * CH : (k + 1) * CH] = qT[
                :, b * CH : (b + 1) * CH
            ]
            wq = (w * q[a * CH : (a + 1) * CH, :]).astype(np.float32)  # [512, NQ]
            wqh, wql = _bf16_split(wq, 2)
            blk = np.concatenate([wqh, wql], axis=1)  # [512, 16]
            QW[:, t * 64 : (t + 1) * 64] = (
                blk.reshape(4, 128, 2 * NQ).transpose(1, 0, 2).reshape(128, 64)
            )
        in_maps.append({"AT": AT, "BT": BT, "QW": QW, "QF": QF})
    return in_maps


def _diag_constant():
    """F value the device computes on the (unmasked) pair-grid diagonal.

    d2 on the diagonal is |err| < 2e-4, and F(x) = erf(sqrt((x+B)/2)) /
    sqrt(x+B) is flat there (variation < 5e-5 relative), so every diagonal
    element lands on the same bf16 value: bf16(F(0)). The bf16 bucket is
    0.4% wide -- vastly wider than the variation -- so this is exact."""
    from scipy.special import erf as _erf

    d = np.sqrt(BIAS)
    c = float(_erf(d / np.sqrt(2.0)) / d)
    return float(np.float32(c).astype(BF16))


def kernel(q, r, cell):
    global LAST_RESULTS
    in_maps = _host_prep(q, r)
    nc = _get_program()
    res = run_bass_kernel_spmd(nc, in_maps, list(range(NCORES)), trace=TRACE)
    LAST_RESULTS = res
    S = sum(float(res.results[c]["OUT"][0, 0]) for c in range(NCORES))
    S -= _diag_constant() * float((q.astype(np.float64) ** 2).sum())
    val = S / TWOPI / 2.0 * NORM_FACTOR
    return np.array([val], np.float32)


# revision 16
# speedup vs baseline: 2.4052x; 1.0360x over previous
"""Trainium2 Bass kernel for the real-space Ewald potential.

Computes  out = NORM/(4*pi) * sum_{i!=j} (q_i . q_j) * erf(|r_i-r_j|/sqrt(2)) / |r_i-r_j|

Strategy (8 NeuronCores, SPMD):
  - The N x N pair grid is split into 8x8 super-tiles of 512x512; core c
    processes row c of the grid, rotated so the diagonal super-tile is the
    core-local tile 0 (identical program, per-core data).
  - d2_ij = s_i + s_j - 2 r_i.r_j comes from ONE K=18 bf16 matmul: r and s
    are split hi/lo (hi/mid/lo for s) into bf16 on the host, so the PE runs
    at full bf16 rate while keeping |d2 err| < 2e-4 (an fp32 matmul runs
    2 passes and is ~2-4x slower).
  - d = sqrt(d2 + 5e-4) and erf(d/sqrt(2)) on the scalar (ACT) engine in
    two strictly separated phases (sqrt and erf live in different ACT
    table sets; interleaving would reload tables at ~2.7us each time).
    1/d via the single-instruction DVE reciprocal_approx_fast in phase 1.
  - F = erf(d/sqrt(2)) * (1/d) multiplies are split between the vector and
    GPSIMD engines (bf16 output); the diagonal is zeroed via a precomputed
    mask on the diagonal super-tile.
  - G[c,i] = sum_j q[j,c] F[j,i] is a K=128 bf16 matmul with q ALSO split
    hi/lo (lhsT [qh|ql], M=16) so q's bf16 rounding cancels; four
    super-tiles share a PSUM bank via PE column-tile quadrants 0/32/64/96.
    The final contraction sum_i q[i,c] G[c,i] is a multiply+reduce on the
    vector engine plus a ones-vector matmul. Each core emits one scalar
    partial; the host sums the 8 partials and applies the constant scale.
"""

import os
import sys

import ml_dtypes
import numpy as np

for _p in ("/opt/trn_rl_repo",):
    if os.path.isdir(_p) and _p not in sys.path:
        sys.path.insert(0, _p)

import concourse.bacc as bacc  # noqa: E402
import concourse.mybir as mybir  # noqa: E402
import concourse.tile as tile  # noqa: E402
from concourse.bass_utils import run_bass_kernel_spmd  # noqa: E402

N = 4096  # atoms
NQ = 8  # charge channels
NCORES = 8
CH = 512  # super-tile edge (i-chunk width / j-chunk height)
NT = 5  # super-tiles per core (1 diagonal + up to 4 off-diagonal x2-weighted)
NGB = (NT + 3) // 4  # G PSUM banks (4 super-tiles per bank)
BIAS = 5e-4  # sqrt(d2 + BIAS): guards bf16-split cancellation (|err| < 2e-4)
INV_SQRT2 = 0.7071067811865476
TWOPI = 2.0 * np.pi
NORM_FACTOR = 90.0474
BF16 = ml_dtypes.bfloat16

# Quadratic-minimax reciprocal constants: with nx = bitcast(~x), t = x*nx
# lands in [-4.5, -4] for any positive fp32 x; 1/x ~ nx*(RA + t*(RB + RC*t))
# to 5.1e-5 relative. Fused with the erf multiply into ONE custom DVE op.
RECIP_A = -0.707106429
RECIP_B = -0.166521999
RECIP_C = -0.013060550


def _register_emul_recip():
    """Register the fused f = in1 * (1/in0) custom DVE op (8 ALU stages)."""
    import concourse.dve_ops as dve_ops
    from concourse.dve_spec import (
        C0,
        C1,
        C2,
        AluOp,
        Bin,
        Spec,
        Src0,
        Src1,
        _has_src1,
        lower as _dve_lower,
    )
    from concourse.dve_uop import DveOpSpec

    name = "EMUL_RECIP_Q_ANT"
    for op in dve_ops.OPS:
        if op.name == name:
            return op

    _nx = Bin(AluOp.BITWISE_NOT, Src0, Src0)
    _t = Src0 * _nx

    def _ref(in0, in1, c0, c1, c2):
        nx = (~np.asarray(in0, np.float32).view(np.int32)).view(np.float32)
        t = in0 * nx
        return ((c0 + t * (c1 + c2 * t)) * nx) * in1

    spec = Spec(body=((C0 + _t * (C1 + C2 * _t)) * _nx) * Src1, reference=_ref)
    row = max(dve_ops._SUB_OPCODE_FOR_NAME.values()) + 1
    assert row < 0x20
    dve_ops._SUB_OPCODE_FOR_NAME[name] = row
    shas = {}
    for ver in ("v3", "v4"):
        s = DveOpSpec(
            name=name, opcode=row, uops=_dve_lower(spec, ver=ver), rd1_en=_has_src1(spec)
        )
        shas[ver] = s.sha(ver)
    op = dve_ops.DveOp(name, spec, subdim=False, uops_sha=shas)
    dve_ops.OPS.append(op)
    dve_ops.CUSTOM_DVE_SPECS[name] = spec
    return op


EMUL_RECIP_Q = _register_emul_recip()

TRACE = bool(os.environ.get("BASS_EWALD_TRACE"))
LAST_RESULTS = None  # BassKernelResults of the most recent run (for test.py)

_prog = None


def _finalize_bank(nc, sp, gk, qf, acc, k):
    OP = mybir.AluOpType
    f32 = mybir.dt.float32
    prod = sp.tile([128, CH], f32, tag=f"prod{k}")
    nc.vector.tensor_tensor(
        prod[:], gk[:], qf[:, k * CH : (k + 1) * CH], OP.mult
    )
    nc.vector.reduce_sum(acc[:, k : k + 1], prod[:], axis=mybir.AxisListType.X)


def _build_program():
    f32 = mybir.dt.float32
    bf16 = mybir.dt.bfloat16
    AF = mybir.ActivationFunctionType
    OP = mybir.AluOpType

    nc = bacc.Bacc("TRN2", target_bir_lowering=False, debug=False, num_devices=NCORES)
    at_d = nc.dram_tensor("AT", [18, NT * CH], bf16, kind="ExternalInput")
    bt_d = nc.dram_tensor("BT", [18, NT * CH], bf16, kind="ExternalInput")
    qw_d = nc.dram_tensor("QW", [128, NT * 64], bf16, kind="ExternalInput")
    qf_d = nc.dram_tensor("QF", [128, NGB * CH], f32, kind="ExternalInput")
    out_d = nc.dram_tensor("OUT", [1, 1], f32, kind="ExternalOutput")

    with tile.TileContext(nc) as tc:
        with (
            tc.tile_pool(name="const", bufs=1) as cp,
            tc.tile_pool(name="work", bufs=3) as wp,
            tc.tile_pool(name="single", bufs=1) as sp,
            tc.tile_pool(name="pd", bufs=2, space="PSUM") as pd,
            tc.tile_pool(name="pg", bufs=1, space="PSUM") as pg,
        ):
            at = cp.tile([18, NT * CH], bf16)
            bt = cp.tile([18, NT * CH], bf16)
            # per-super-tile chunks on two DMA queues: tile 0's operands land
            # first so the first d2 matmul isn't gated on the whole load.
            for t in range(NT):
                sl = slice(t * CH, (t + 1) * CH)
                nc.sync.dma_start(at[:, sl], at_d[:, sl])
                nc.gpsimd.dma_start(bt[:, sl], bt_d[:, sl])
            qw = cp.tile([128, NT * 64], bf16)
            nc.gpsimd.dma_start(qw[:], qw_d[:])
            qf = cp.tile([128, NGB * CH], f32)
            dall = cp.tile([128, NT * 2048], f32)
            ones = cp.tile([128, 1], f32)
            nc.vector.memset(ones[:], 1.0)
            bias_t = cp.tile([128, 1], f32)
            nc.vector.memset(bias_t[:], BIAS)
            gbanks = []
            for k in range(NGB):
                gk = pg.tile([128, CH], f32, tag=f"g{k}")
                nc.vector.memset(gk[:], 0.0)
                gbanks.append(gk)

            # Phase 1: d2 matmuls + sqrt (sqrt ACT table set) + 1/d on DVE.
            for t in range(NT):
                for h in (0, 1):
                    ps = pd.tile([128, 1024], f32, tag="d2")
                    for u in (0, 1):
                        jb = 2 * h + u
                        nc.tensor.matmul(
                            ps[:, u * CH : (u + 1) * CH],
                            bt[:, t * CH + jb * 128 : t * CH + (jb + 1) * 128],
                            at[:, t * CH : (t + 1) * CH],
                            start=True,
                            stop=True,
                        )
                    dsl = dall[:, (2 * t + h) * 1024 : (2 * t + h + 1) * 1024]
                    nc.scalar.activation(dsl, ps[:], AF.Sqrt, bias=bias_t[:])

            # qf is only needed by the finalize stage; issuing its DMA after
            # phase 1 keeps the head of the sync queue clear for AT/BT.
            nc.gpsimd.dma_start(qf[:], qf_d[:])

            # Keep the two ACT table sets in disjoint program ranges.
            tc.no_sync_barrier()

            # Phase 2: erf + F-multiply (+ diagonal mask) + G matmuls.
            acc = sp.tile([128, NGB], f32, tag="acc")
            for t in range(NT):
                k, m = divmod(t, 4)  # G bank, quadrant
                et = wp.tile([128, 2048], f32, tag="e")
                nc.scalar.activation(
                    et[:], dall[:, t * 2048 : (t + 1) * 2048], AF.Erf, scale=INV_SQRT2
                )
                for h in (0, 1):
                    esl = et[:, h * 1024 : (h + 1) * 1024]
                    dsl = dall[:, (2 * t + h) * 1024 : (2 * t + h + 1) * 1024]
                    f = wp.tile([128, 1024], bf16, tag="f")
                    nc.vector._custom_dve(
                        EMUL_RECIP_Q,
                        out=f[:],
                        in0=dsl,
                        in1=esl,
                        s0=RECIP_A,
                        s1=RECIP_B,
                        imm2=RECIP_C,
                    )
                    for u in (0, 1):
                        jb = 2 * h + u
                        nc.tensor.matmul(
                            gbanks[k][32 * m : 32 * m + 16, :],
                            qw[:, t * 64 + jb * 16 : t * 64 + (jb + 1) * 16],
                            f[:, u * CH : (u + 1) * CH],
                            start=(jb == 0),
                            stop=(jb == 3),
                            tile_position=(0, 32 * m),
                        )

            # Finalize: per G bank, multiply+reduce; then sum partitions.
            for k in range(NGB):
                _finalize_bank(nc, sp, gbanks[k], qf, acc, k)
            accsum = sp.tile([128, 1], f32, tag="accsum")
            nc.vector.reduce_sum(accsum[:], acc[:], axis=mybir.AxisListType.X)
            tot = pg.tile([1, 1], f32, tag="tot")
            nc.tensor.matmul(tot[:], accsum[:], ones[:], start=True, stop=True)
            res = sp.tile([1, 1], f32, tag="res")
            nc.scalar.copy(res[:], tot[:])
            nc.sync.dma_start(out_d[:], res[:])

    nc.compile()
    return nc


def _get_program():
    global _prog
    if _prog is None:
        _prog = _build_program()
    return _prog


def _bf16_split(x32, parts):
    """Split fp32 array into `parts` bf16 arrays summing to x32 (greedy)."""
    out = []
    rem = x32.astype(np.float64)
    for _ in range(parts):
        p = rem.astype(np.float32).astype(BF16)
        out.append(p)
        rem = rem - p.astype(np.float64)
    return out


def _host_prep(q, r):
    q = np.ascontiguousarray(np.asarray(q, np.float32))
    r = np.ascontiguousarray(np.asarray(r, np.float32))
    r64 = r.astype(np.float64)
    s64 = (r64 * r64).sum(1)

    rh, rl = _bf16_split(r, 2)  # [N,3] bf16 each
    m2rh, m2rl = (-2.0 * rh.astype(np.float32)).astype(BF16), (
        -2.0 * rl.astype(np.float32)
    ).astype(BF16)
    sh, sm, sl = _bf16_split(s64, 3)  # [N] bf16 each
    onesN = np.ones(N, BF16)

    # rhs rows (i side) pair with lhsT rows (j side), K=18:
    #   -2rh_j*rh_i, -2rh_j*rl_i, -2rl_j*rh_i, -2rl_j*rl_i (12 rows),
    #   (sh+sm+sl)_j * 1 (3 rows), 1 * (sh+sm+sl)_i (3 rows)
    A18 = np.concatenate(
        [rh.T, rl.T, rh.T, rl.T, [onesN, onesN, onesN], [sh, sm, sl]]
    ).astype(BF16)  # [18, N]
    B18 = np.concatenate(
        [m2rh.T, m2rh.T, m2rl.T, m2rl.T, [sh, sm, sl], [onesN, onesN, onesN]]
    ).astype(BF16)  # [18, N]

    qT = np.ascontiguousarray(q.T)  # [NQ, N] f32

    # 36 super-tiles of the symmetric pair grid: 8 diagonal (w=1, core-local
    # tile 0, diag-masked) + 28 upper-triangle pairs (w=2), dealt round-robin;
    # cores with only 3 pairs get a zero-weight dummy tile.
    pairs = [(a, b) for a in range(8) for b in range(a + 1, 8)]
    assignments = [[(c, c, 1.0)] for c in range(NCORES)]
    for idx, (a, b) in enumerate(pairs):
        assignments[idx % NCORES].append((a, b, 2.0))
    for c in range(NCORES):
        while len(assignments[c]) < NT:
            assignments[c].append((c, c, 0.0))

    in_maps = []
    for c in range(NCORES):
        tiles = assignments[c]  # (j-chunk a, i-chunk b, weight)
        AT = np.empty((18, NT * CH), BF16)
        BT = np.empty((18, NT * CH), BF16)
        QW = np.empty((128, NT * 64), BF16)
        QF = np.zeros((128, NGB * CH), np.float32)
        for t, (a, b, w) in enumerate(tiles):
            k, m = divmod(t, 4)
            AT[:, t * CH : (t + 1) * CH] = A18[:, b * CH : (b + 1) * CH]
            BT[:, t * CH : (t + 1) * CH] = B18[:, a * CH : (a + 1) * CH]
            # Finalize reads quadrant rows 32m + [0..16): both the qh and ql
            # halves of G contract against the same fp32 qT chunk.
            QF[32 * m : 32 * m + NQ, k * CH : (k + 1) * CH] = qT[
                :, b * CH : (b + 1) * CH
            ]
            QF[32 * m + NQ : 32 * m + 2 * NQ, k * CH : (k + 1) * CH] = qT[
                :, b * CH : (b + 1) * CH
            ]
            wq = (w * q[a * CH : (a + 1) * CH, :]).astype(np.float32)  # [512, NQ]
            wqh, wql = _bf16_split(wq, 2)
            blk = np.concatenate([wqh, wql], axis=1)  # [512, 16]
            QW[:, t * 64 : (t + 1) * 64] = (
                blk.reshape(4, 128, 2 * NQ).transpose(1, 0, 2).reshape(128, 64)
            )
        in_maps.append({"AT": AT, "BT": BT, "QW": QW, "QF": QF})
    return in_maps


def _diag_constant():
    """F value the device computes on the (unmasked) pair-grid diagonal.

    d2 on the diagonal is |err| < 2e-4, and F(x) = erf(sqrt((x+B)/2)) /
    sqrt(x+B) is flat there (variation < 1e-4 relative), so every diagonal
    element lands on the same bf16 value: bf16 applied to the device's
    erf * quadratic-reciprocal product at d = sqrt(BIAS). The bf16 bucket
    is 0.4% wide -- vastly wider than the variation -- so this is exact."""
    from scipy.special import erf as _erf

    d0 = np.float32(np.sqrt(BIAS))
    e0 = np.float32(_erf(float(d0) * INV_SQRT2))
    nx = (~d0.reshape(1).view(np.int32)).view(np.float32)[0]
    t = np.float32(d0 * nx)
    rq = np.float32(
        (np.float32(RECIP_A) + t * (np.float32(RECIP_B) + np.float32(RECIP_C) * t))
        * nx
    )
    return float((e0 * rq).astype(BF16))


def kernel(q, r, cell):
    global LAST_RESULTS
    in_maps = _host_prep(q, r)
    nc = _get_program()
    res = run_bass_kernel_spmd(nc, in_maps, list(range(NCORES)), trace=TRACE)
    LAST_RESULTS = res
    S = sum(float(res.results[c]["OUT"][0, 0]) for c in range(NCORES))
    S -= _diag_constant() * float((q.astype(np.float64) ** 2).sum())
    val = S / TWOPI / 2.0 * NORM_FACTOR
    return np.array([val], np.float32)


# revision 17
# speedup vs baseline: 2.4381x; 1.0137x over previous
"""Trainium2 Bass kernel for the real-space Ewald potential.

Computes  out = NORM/(4*pi) * sum_{i!=j} (q_i . q_j) * erf(|r_i-r_j|/sqrt(2)) / |r_i-r_j|

Strategy (8 NeuronCores, SPMD):
  - The N x N pair grid is split into 8x8 super-tiles of 512x512; core c
    processes row c of the grid, rotated so the diagonal super-tile is the
    core-local tile 0 (identical program, per-core data).
  - d2_ij = s_i + s_j - 2 r_i.r_j comes from ONE K=18 bf16 matmul: r and s
    are split hi/lo (hi/mid/lo for s) into bf16 on the host, so the PE runs
    at full bf16 rate while keeping |d2 err| < 2e-4 (an fp32 matmul runs
    2 passes and is ~2-4x slower).
  - d = sqrt(d2 + 5e-4) and erf(d/sqrt(2)) on the scalar (ACT) engine in
    two strictly separated phases (sqrt and erf live in different ACT
    table sets; interleaving would reload tables at ~2.7us each time).
    1/d via the single-instruction DVE reciprocal_approx_fast in phase 1.
  - F = erf(d/sqrt(2)) * (1/d) multiplies are split between the vector and
    GPSIMD engines (bf16 output); the diagonal is zeroed via a precomputed
    mask on the diagonal super-tile.
  - G[c,i] = sum_j q[j,c] F[j,i] is a K=128 bf16 matmul with q ALSO split
    hi/lo (lhsT [qh|ql], M=16) so q's bf16 rounding cancels; four
    super-tiles share a PSUM bank via PE column-tile quadrants 0/32/64/96.
    The final contraction sum_i q[i,c] G[c,i] is a multiply+reduce on the
    vector engine plus a ones-vector matmul. Each core emits one scalar
    partial; the host sums the 8 partials and applies the constant scale.
"""

import os
import sys

import ml_dtypes
import numpy as np

for _p in ("/opt/trn_rl_repo",):
    if os.path.isdir(_p) and _p not in sys.path:
        sys.path.insert(0, _p)

import concourse.bacc as bacc  # noqa: E402
import concourse.mybir as mybir  # noqa: E402
import concourse.tile as tile  # noqa: E402
from concourse.bass_utils import run_bass_kernel_spmd  # noqa: E402

N = 4096  # atoms
NQ = 8  # charge channels
NCORES = 8
CH = 512  # super-tile edge (i-chunk width / j-chunk height)
NT = 5  # super-tiles per core (1 diagonal + up to 4 off-diagonal x2-weighted)
NGB = (NT + 3) // 4  # G PSUM banks (4 super-tiles per bank)
BIAS = 5e-4  # sqrt(d2 + BIAS): guards bf16-split cancellation (|err| < 2e-4)
INV_SQRT2 = 0.7071067811865476
TWOPI = 2.0 * np.pi
NORM_FACTOR = 90.0474
BF16 = ml_dtypes.bfloat16

# Quadratic-minimax reciprocal constants: with nx = bitcast(~x), t = x*nx
# lands in [-4.5, -4] for any positive fp32 x; 1/x ~ nx*(RA + t*(RB + RC*t))
# to 5.1e-5 relative. Fused with the erf multiply into ONE custom DVE op.
RECIP_A = -0.707106429
RECIP_B = -0.166521999
RECIP_C = -0.013060550


def _register_emul_recip():
    """Register the fused f = in1 * (1/in0) custom DVE op (8 ALU stages)."""
    import concourse.dve_ops as dve_ops
    from concourse.dve_spec import (
        C0,
        C1,
        C2,
        AluOp,
        Bin,
        Spec,
        Src0,
        Src1,
        _has_src1,
        lower as _dve_lower,
    )
    from concourse.dve_uop import DveOpSpec

    name = "EMUL_RECIP_Q_ANT"
    for op in dve_ops.OPS:
        if op.name == name:
            return op

    _nx = Bin(AluOp.BITWISE_NOT, Src0, Src0)
    _t = Src0 * _nx

    def _ref(in0, in1, c0, c1, c2):
        nx = (~np.asarray(in0, np.float32).view(np.int32)).view(np.float32)
        t = in0 * nx
        return ((c0 + t * (c1 + c2 * t)) * nx) * in1

    spec = Spec(body=((C0 + _t * (C1 + C2 * _t)) * _nx) * Src1, reference=_ref)
    row = max(dve_ops._SUB_OPCODE_FOR_NAME.values()) + 1
    assert row < 0x20
    dve_ops._SUB_OPCODE_FOR_NAME[name] = row
    shas = {}
    for ver in ("v3", "v4"):
        s = DveOpSpec(
            name=name, opcode=row, uops=_dve_lower(spec, ver=ver), rd1_en=_has_src1(spec)
        )
        shas[ver] = s.sha(ver)
    op = dve_ops.DveOp(name, spec, subdim=False, uops_sha=shas)
    dve_ops.OPS.append(op)
    dve_ops.CUSTOM_DVE_SPECS[name] = spec
    return op


EMUL_RECIP_Q = _register_emul_recip()

TRACE = bool(os.environ.get("BASS_EWALD_TRACE"))
LAST_RESULTS = None  # BassKernelResults of the most recent run (for test.py)

_prog = None


def _finalize_bank(nc, sp, gk, qf, acc, k):
    OP = mybir.AluOpType
    f32 = mybir.dt.float32
    prod = sp.tile([128, CH], f32, tag=f"prod{k}")
    nc.vector.tensor_tensor(
        prod[:], gk[:], qf[:, k * CH : (k + 1) * CH], OP.mult
    )
    nc.vector.reduce_sum(acc[:, k : k + 1], prod[:], axis=mybir.AxisListType.X)


def _build_program():
    f32 = mybir.dt.float32
    bf16 = mybir.dt.bfloat16
    AF = mybir.ActivationFunctionType
    OP = mybir.AluOpType

    nc = bacc.Bacc("TRN2", target_bir_lowering=False, debug=False, num_devices=NCORES)
    at_d = nc.dram_tensor("AT", [18, NT * CH], bf16, kind="ExternalInput")
    bt_d = nc.dram_tensor("BT", [18, NT * CH], bf16, kind="ExternalInput")
    qw_d = nc.dram_tensor("QW", [128, NT * 64], bf16, kind="ExternalInput")
    qf_d = nc.dram_tensor("QF", [128, NGB * CH], f32, kind="ExternalInput")
    out_d = nc.dram_tensor("OUT", [1, 1], f32, kind="ExternalOutput")

    with tile.TileContext(nc) as tc:
        with (
            tc.tile_pool(name="const", bufs=1) as cp,
            tc.tile_pool(name="work", bufs=3) as wp,
            tc.tile_pool(name="single", bufs=1) as sp,
            tc.tile_pool(name="pd", bufs=2, space="PSUM") as pd,
            tc.tile_pool(name="pg", bufs=1, space="PSUM") as pg,
        ):
            at = cp.tile([18, NT * CH], bf16)
            bt = cp.tile([18, NT * CH], bf16)
            # per-super-tile chunks on two DMA queues: tile 0's operands land
            # first so the first d2 matmul isn't gated on the whole load.
            for t in range(NT):
                sl = slice(t * CH, (t + 1) * CH)
                nc.sync.dma_start(at[:, sl], at_d[:, sl])
                nc.gpsimd.dma_start(bt[:, sl], bt_d[:, sl])
            qw = cp.tile([128, NT * 64], bf16)
            nc.gpsimd.dma_start(qw[:], qw_d[:])
            qf = cp.tile([128, NGB * CH], f32)
            dall = cp.tile([128, NT * 2048], f32)
            ones = cp.tile([128, 1], f32)
            nc.vector.memset(ones[:], 1.0)
            bias_t = cp.tile([128, 1], f32)
            nc.vector.memset(bias_t[:], BIAS)
            gbanks = []
            for k in range(NGB):
                gk = pg.tile([128, CH], f32, tag=f"g{k}")
                nc.vector.memset(gk[:], 0.0)
                gbanks.append(gk)

            # Phase 1: d2 matmuls + sqrt (sqrt ACT table set) + 1/d on DVE.
            for t in range(NT):
                for h in (0, 1):
                    ps = pd.tile([128, 1024], f32, tag="d2")
                    for u in (0, 1):
                        jb = 2 * h + u
                        nc.tensor.matmul(
                            ps[:, u * CH : (u + 1) * CH],
                            bt[:, t * CH + jb * 128 : t * CH + (jb + 1) * 128],
                            at[:, t * CH : (t + 1) * CH],
                            start=True,
                            stop=True,
                        )
                    dsl = dall[:, (2 * t + h) * 1024 : (2 * t + h + 1) * 1024]
                    nc.scalar.activation(dsl, ps[:], AF.Sqrt, bias=bias_t[:])

            # qf is only needed by the finalize stage; issuing its DMA after
            # phase 1 keeps the head of the sync queue clear for AT/BT.
            nc.gpsimd.dma_start(qf[:], qf_d[:])

            # Keep the two ACT table sets in disjoint program ranges.
            tc.no_sync_barrier()

            # Phase 2: erf + F-multiply (+ diagonal mask) + G matmuls.
            acc = sp.tile([128, NGB], f32, tag="acc")
            for t in range(NT):
                k, m = divmod(t, 4)  # G bank, quadrant
                et = wp.tile([128, 2048], f32, tag="e")
                nc.scalar.activation(
                    et[:], dall[:, t * 2048 : (t + 1) * 2048], AF.Erf, scale=INV_SQRT2
                )
                for h in (0, 1):
                    esl = et[:, h * 1024 : (h + 1) * 1024]
                    dsl = dall[:, (2 * t + h) * 1024 : (2 * t + h + 1) * 1024]
                    f = wp.tile([128, 1024], bf16, tag="f")
                    nc.vector._custom_dve(
                        EMUL_RECIP_Q,
                        out=f[:],
                        in0=dsl,
                        in1=esl,
                        s0=RECIP_A,
                        s1=RECIP_B,
                        imm2=RECIP_C,
                    )
                    for u in (0, 1):
                        jb = 2 * h + u
                        nc.tensor.matmul(
                            gbanks[k][32 * m : 32 * m + 16, :],
                            qw[:, t * 64 + jb * 16 : t * 64 + (jb + 1) * 16],
                            f[:, u * CH : (u + 1) * CH],
                            start=(jb == 0),
                            stop=(jb == 3),
                            tile_position=(0, 32 * m),
                        )
                if t == 3:
                    # bank 0 complete: overlap its finalize with tile 4's work
                    _finalize_bank(nc, sp, gbanks[0], qf, acc, 0)

            # Finalize remaining banks; then sum partitions.
            for k in range(1, NGB):
                _finalize_bank(nc, sp, gbanks[k], qf, acc, k)
            accsum = sp.tile([128, 1], f32, tag="accsum")
            nc.vector.reduce_sum(accsum[:], acc[:], axis=mybir.AxisListType.X)
            tot = pg.tile([1, 1], f32, tag="tot")
            nc.tensor.matmul(tot[:], accsum[:], ones[:], start=True, stop=True)
            res = sp.tile([1, 1], f32, tag="res")
            nc.scalar.copy(res[:], tot[:])
            nc.sync.dma_start(out_d[:], res[:])

    nc.compile()
    return nc


def _get_program():
    global _prog
    if _prog is None:
        _prog = _build_program()
    return _prog


def _bf16_split(x32, parts):
    """Split fp32 array into `parts` bf16 arrays summing to x32 (greedy)."""
    out = []
    rem = x32.astype(np.float64)
    for _ in range(parts):
        p = rem.astype(np.float32).astype(BF16)
        out.append(p)
        rem = rem - p.astype(np.float64)
    return out


def _host_prep(q, r):
    q = np.ascontiguousarray(np.asarray(q, np.float32))
    r = np.ascontiguousarray(np.asarray(r, np.float32))
    r64 = r.astype(np.float64)
    s64 = (r64 * r64).sum(1)

    rh, rl = _bf16_split(r, 2)  # [N,3] bf16 each
    m2rh, m2rl = (-2.0 * rh.astype(np.float32)).astype(BF16), (
        -2.0 * rl.astype(np.float32)
    ).astype(BF16)
    sh, sm, sl = _bf16_split(s64, 3)  # [N] bf16 each
    onesN = np.ones(N, BF16)

    # rhs rows (i side) pair with lhsT rows (j side), K=18:
    #   -2rh_j*rh_i, -2rh_j*rl_i, -2rl_j*rh_i, -2rl_j*rl_i (12 rows),
    #   (sh+sm+sl)_j * 1 (3 rows), 1 * (sh+sm+sl)_i (3 rows)
    A18 = np.concatenate(
        [rh.T, rl.T, rh.T, rl.T, [onesN, onesN, onesN], [sh, sm, sl]]
    ).astype(BF16)  # [18, N]
    B18 = np.concatenate(
        [m2rh.T, m2rh.T, m2rl.T, m2rl.T, [sh, sm, sl], [onesN, onesN, onesN]]
    ).astype(BF16)  # [18, N]

    qT = np.ascontiguousarray(q.T)  # [NQ, N] f32

    # 36 super-tiles of the symmetric pair grid: 8 diagonal (w=1, core-local
    # tile 0, diag-masked) + 28 upper-triangle pairs (w=2), dealt round-robin;
    # cores with only 3 pairs get a zero-weight dummy tile.
    pairs = [(a, b) for a in range(8) for b in range(a + 1, 8)]
    assignments = [[(c, c, 1.0)] for c in range(NCORES)]
    for idx, (a, b) in enumerate(pairs):
        assignments[idx % NCORES].append((a, b, 2.0))
    for c in range(NCORES):
        while len(assignments[c]) < NT:
            assignments[c].append((c, c, 0.0))

    in_maps = []
    for c in range(NCORES):
        tiles = assignments[c]  # (j-chunk a, i-chunk b, weight)
        AT = np.empty((18, NT * CH), BF16)
        BT = np.empty((18, NT * CH), BF16)
        QW = np.empty((128, NT * 64), BF16)
        QF = np.zeros((128, NGB * CH), np.float32)
        for t, (a, b, w) in enumerate(tiles):
            k, m = divmod(t, 4)
            AT[:, t * CH : (t + 1) * CH] = A18[:, b * CH : (b + 1) * CH]
            BT[:, t * CH : (t + 1) * CH] = B18[:, a * CH : (a + 1) * CH]
            # Finalize reads quadrant rows 32m + [0..16): both the qh and ql
            # halves of G contract against the same fp32 qT chunk.
            QF[32 * m : 32 * m + NQ, k * CH : (k + 1) * CH] = qT[
                :, b * CH : (b + 1) * CH
            ]
            QF[32 * m + NQ : 32 * m + 2 * NQ, k * CH : (k + 1) * CH] = qT[
                :, b * CH : (b + 1) * CH
            ]
            wq = (w * q[a * CH : (a + 1) * CH, :]).astype(np.float32)  # [512, NQ]
            wqh, wql = _bf16_split(wq, 2)
            blk = np.concatenate([wqh, wql], axis=1)  # [512, 16]
            QW[:, t * 64 : (t + 1) * 64] = (
                blk.reshape(4, 128, 2 * NQ).transpose(1, 0, 2).reshape(128, 64)
            )
        in_maps.append({"AT": AT, "BT": BT, "QW": QW, "QF": QF})
    return in_maps


def _diag_constant():
    """F value the device computes on the (unmasked) pair-grid diagonal.

    d2 on the diagonal is |err| < 2e-4, and F(x) = erf(sqrt((x+B)/2)) /
    sqrt(x+B) is flat there (variation < 1e-4 relative), so every diagonal
    element lands on the same bf16 value: bf16 applied to the device's
    erf * quadratic-reciprocal product at d = sqrt(BIAS). The bf16 bucket
    is 0.4% wide -- vastly wider than the variation -- so this is exact."""
    from scipy.special import erf as _erf

    d0 = np.float32(np.sqrt(BIAS))
    e0 = np.float32(_erf(float(d0) * INV_SQRT2))
    nx = (~d0.reshape(1).view(np.int32)).view(np.float32)[0]
    t = np.float32(d0 * nx)
    rq = np.float32(
        (np.float32(RECIP_A) + t * (np.float32(RECIP_B) + np.float32(RECIP_C) * t))
        * nx
    )
    return float((e0 * rq).astype(BF16))


def kernel(q, r, cell):
    global LAST_RESULTS
    in_maps = _host_prep(q, r)
    nc = _get_program()
    res = run_bass_kernel_spmd(nc, in_maps, list(range(NCORES)), trace=TRACE)
    LAST_RESULTS = res
    S = sum(float(res.results[c]["OUT"][0, 0]) for c in range(NCORES))
    S -= _diag_constant() * float((q.astype(np.float64) ** 2).sum())
    val = S / TWOPI / 2.0 * NORM_FACTOR
    return np.array([val], np.float32)


# revision 18
# speedup vs baseline: 2.4525x; 1.0059x over previous
"""Trainium2 Bass kernel for the real-space Ewald potential.

Computes  out = NORM/(4*pi) * sum_{i!=j} (q_i . q_j) * erf(|r_i-r_j|/sqrt(2)) / |r_i-r_j|

Strategy (8 NeuronCores, SPMD):
  - The N x N pair grid is split into 8x8 super-tiles of 512x512; core c
    processes row c of the grid, rotated so the diagonal super-tile is the
    core-local tile 0 (identical program, per-core data).
  - d2_ij = s_i + s_j - 2 r_i.r_j comes from ONE K=18 bf16 matmul: r and s
    are split hi/lo (hi/mid/lo for s) into bf16 on the host, so the PE runs
    at full bf16 rate while keeping |d2 err| < 2e-4 (an fp32 matmul runs
    2 passes and is ~2-4x slower).
  - d = sqrt(d2 + 5e-4) and erf(d/sqrt(2)) on the scalar (ACT) engine in
    two strictly separated phases (sqrt and erf live in different ACT
    table sets; interleaving would reload tables at ~2.7us each time).
    1/d via the single-instruction DVE reciprocal_approx_fast in phase 1.
  - F = erf(d/sqrt(2)) * (1/d) multiplies are split between the vector and
    GPSIMD engines (bf16 output); the diagonal is zeroed via a precomputed
    mask on the diagonal super-tile.
  - G[c,i] = sum_j q[j,c] F[j,i] is a K=128 bf16 matmul with q ALSO split
    hi/lo (lhsT [qh|ql], M=16) so q's bf16 rounding cancels; four
    super-tiles share a PSUM bank via PE column-tile quadrants 0/32/64/96.
    The final contraction sum_i q[i,c] G[c,i] is a multiply+reduce on the
    vector engine plus a ones-vector matmul. Each core emits one scalar
    partial; the host sums the 8 partials and applies the constant scale.
"""

import os
import sys

import ml_dtypes
import numpy as np

for _p in ("/opt/trn_rl_repo",):
    if os.path.isdir(_p) and _p not in sys.path:
        sys.path.insert(0, _p)

import concourse.bacc as bacc  # noqa: E402
import concourse.mybir as mybir  # noqa: E402
import concourse.tile as tile  # noqa: E402
from concourse.bass_utils import run_bass_kernel_spmd  # noqa: E402

N = 4096  # atoms
NQ = 8  # charge channels
NCORES = 8
CH = 512  # super-tile edge (i-chunk width / j-chunk height)
NT = 5  # super-tiles per core (1 diagonal + up to 4 off-diagonal x2-weighted)
NGB = (NT + 3) // 4  # G PSUM banks (4 super-tiles per bank)
BIAS = 5e-4  # sqrt(d2 + BIAS): guards bf16-split cancellation (|err| < 2e-4)
INV_SQRT2 = 0.7071067811865476
TWOPI = 2.0 * np.pi
NORM_FACTOR = 90.0474
BF16 = ml_dtypes.bfloat16

# Quadratic-minimax reciprocal constants: with nx = bitcast(~x), t = x*nx
# lands in [-4.5, -4] for any positive fp32 x; 1/x ~ nx*(RA + t*(RB + RC*t))
# to 5.1e-5 relative. Fused with the erf multiply into ONE custom DVE op.
RECIP_A = -0.707106429
RECIP_B = -0.166521999
RECIP_C = -0.013060550


def _register_emul_recip():
    """Register the fused f = in1 * (1/in0) custom DVE op (8 ALU stages)."""
    import concourse.dve_ops as dve_ops
    from concourse.dve_spec import (
        C0,
        C1,
        C2,
        AluOp,
        Bin,
        Spec,
        Src0,
        Src1,
        _has_src1,
        lower as _dve_lower,
    )
    from concourse.dve_uop import DveOpSpec

    name = "EMUL_RECIP_Q_ANT"
    for op in dve_ops.OPS:
        if op.name == name:
            return op

    _nx = Bin(AluOp.BITWISE_NOT, Src0, Src0)
    _t = Src0 * _nx

    def _ref(in0, in1, c0, c1, c2):
        nx = (~np.asarray(in0, np.float32).view(np.int32)).view(np.float32)
        t = in0 * nx
        return ((c0 + t * (c1 + c2 * t)) * nx) * in1

    spec = Spec(body=((C0 + _t * (C1 + C2 * _t)) * _nx) * Src1, reference=_ref)
    row = max(dve_ops._SUB_OPCODE_FOR_NAME.values()) + 1
    assert row < 0x20
    dve_ops._SUB_OPCODE_FOR_NAME[name] = row
    shas = {}
    for ver in ("v3", "v4"):
        s = DveOpSpec(
            name=name, opcode=row, uops=_dve_lower(spec, ver=ver), rd1_en=_has_src1(spec)
        )
        shas[ver] = s.sha(ver)
    op = dve_ops.DveOp(name, spec, subdim=False, uops_sha=shas)
    dve_ops.OPS.append(op)
    dve_ops.CUSTOM_DVE_SPECS[name] = spec
    return op


EMUL_RECIP_Q = _register_emul_recip()

# Super-tiles whose 1/d is precomputed on the DVE during phase 1 (hidden
# under the ACT sqrt pass) and converted to bf16, so their phase-2 multiply
# runs at the DVE 2x bf16 rate. The last tiles take this cheap path so the
# DVE tail of phase 2 shrinks; tile 0/1 keep the fused 1x custom op.
BF_PATH_T = (2, 3, 4)

TRACE = bool(os.environ.get("BASS_EWALD_TRACE"))
LAST_RESULTS = None  # BassKernelResults of the most recent run (for test.py)

_prog = None


def _finalize_bank(nc, sp, gk, qf, acc, k):
    OP = mybir.AluOpType
    f32 = mybir.dt.float32
    prod = sp.tile([128, CH], f32, tag=f"prod{k}")
    nc.vector.tensor_tensor(
        prod[:], gk[:], qf[:, k * CH : (k + 1) * CH], OP.mult
    )
    nc.vector.reduce_sum(acc[:, k : k + 1], prod[:], axis=mybir.AxisListType.X)


def _build_program():
    f32 = mybir.dt.float32
    bf16 = mybir.dt.bfloat16
    AF = mybir.ActivationFunctionType
    OP = mybir.AluOpType

    nc = bacc.Bacc("TRN2", target_bir_lowering=False, debug=False, num_devices=NCORES)
    at_d = nc.dram_tensor("AT", [18, NT * CH], bf16, kind="ExternalInput")
    bt_d = nc.dram_tensor("BT", [18, NT * CH], bf16, kind="ExternalInput")
    qw_d = nc.dram_tensor("QW", [128, NT * 64], bf16, kind="ExternalInput")
    qf_d = nc.dram_tensor("QF", [128, NGB * CH], f32, kind="ExternalInput")
    out_d = nc.dram_tensor("OUT", [1, 1], f32, kind="ExternalOutput")

    with tile.TileContext(nc) as tc:
        with (
            tc.tile_pool(name="const", bufs=1) as cp,
            tc.tile_pool(name="work", bufs=3) as wp,
            tc.tile_pool(name="single", bufs=1) as sp,
            tc.tile_pool(name="pd", bufs=2, space="PSUM") as pd,
            tc.tile_pool(name="pg", bufs=1, space="PSUM") as pg,
        ):
            at = cp.tile([18, NT * CH], bf16)
            bt = cp.tile([18, NT * CH], bf16)
            # per-super-tile chunks on two DMA queues: tile 0's operands land
            # first so the first d2 matmul isn't gated on the whole load.
            for t in range(NT):
                sl = slice(t * CH, (t + 1) * CH)
                nc.sync.dma_start(at[:, sl], at_d[:, sl])
                nc.gpsimd.dma_start(bt[:, sl], bt_d[:, sl])
            qw = cp.tile([128, NT * 64], bf16)
            nc.gpsimd.dma_start(qw[:], qw_d[:])
            qf = cp.tile([128, NGB * CH], f32)
            dall = cp.tile([128, NT * 2048], f32)
            rdb = cp.tile([128, len(BF_PATH_T) * 2048], bf16)
            ones = cp.tile([128, 1], f32)
            nc.vector.memset(ones[:], 1.0)
            bias_t = cp.tile([128, 1], f32)
            nc.vector.memset(bias_t[:], BIAS)
            gbanks = []
            for k in range(NGB):
                gk = pg.tile([128, CH], f32, tag=f"g{k}")
                nc.vector.memset(gk[:], 0.0)
                gbanks.append(gk)

            # Phase 1: d2 matmuls + sqrt (sqrt ACT table set) + 1/d on DVE.
            for t in range(NT):
                for h in (0, 1):
                    ps = pd.tile([128, 1024], f32, tag="d2")
                    for u in (0, 1):
                        jb = 2 * h + u
                        nc.tensor.matmul(
                            ps[:, u * CH : (u + 1) * CH],
                            bt[:, t * CH + jb * 128 : t * CH + (jb + 1) * 128],
                            at[:, t * CH : (t + 1) * CH],
                            start=True,
                            stop=True,
                        )
                    dsl = dall[:, (2 * t + h) * 1024 : (2 * t + h + 1) * 1024]
                    nc.scalar.activation(dsl, ps[:], AF.Sqrt, bias=bias_t[:])
                    if t in BF_PATH_T:
                        u16 = 2 * BF_PATH_T.index(t) + h
                        rd32 = wp.tile([128, 1024], f32, tag="rd32")
                        nc.vector.reciprocal_approx_fast(out=rd32[:], in_=dsl)
                        nc.vector.tensor_copy(
                            rdb[:, u16 * 1024 : (u16 + 1) * 1024], rd32[:]
                        )

            # qf is only needed by the finalize stage; issuing its DMA after
            # phase 1 keeps the head of the sync queue clear for AT/BT.
            nc.gpsimd.dma_start(qf[:], qf_d[:])

            # Keep the two ACT table sets in disjoint program ranges.
            tc.no_sync_barrier()

            # Phase 2: erf + F-multiply (+ diagonal mask) + G matmuls.
            acc = sp.tile([128, NGB], f32, tag="acc")
            for t in range(NT):
                k, m = divmod(t, 4)  # G bank, quadrant
                bfp = t in BF_PATH_T
                et = wp.tile([128, 2048], bf16 if bfp else f32, tag="e16" if bfp else "e32")
                nc.scalar.activation(
                    et[:], dall[:, t * 2048 : (t + 1) * 2048], AF.Erf, scale=INV_SQRT2
                )
                for h in (0, 1):
                    esl = et[:, h * 1024 : (h + 1) * 1024]
                    f = wp.tile([128, 1024], bf16, tag="f")
                    if bfp:
                        u16 = 2 * BF_PATH_T.index(t) + h
                        nc.vector.tensor_tensor(
                            f[:], esl, rdb[:, u16 * 1024 : (u16 + 1) * 1024], OP.mult
                        )
                    else:
                        dsl = dall[:, (2 * t + h) * 1024 : (2 * t + h + 1) * 1024]
                        nc.vector._custom_dve(
                            EMUL_RECIP_Q,
                            out=f[:],
                            in0=dsl,
                            in1=esl,
                            s0=RECIP_A,
                            s1=RECIP_B,
                            imm2=RECIP_C,
                        )
                    for u in (0, 1):
                        jb = 2 * h + u
                        nc.tensor.matmul(
                            gbanks[k][32 * m : 32 * m + 16, :],
                            qw[:, t * 64 + jb * 16 : t * 64 + (jb + 1) * 16],
                            f[:, u * CH : (u + 1) * CH],
                            start=(jb == 0),
                            stop=(jb == 3),
                            tile_position=(0, 32 * m),
                        )
                if t == 3:
                    # bank 0 complete: overlap its finalize with tile 4's work
                    _finalize_bank(nc, sp, gbanks[0], qf, acc, 0)

            # Finalize remaining banks; then sum partitions.
            for k in range(1, NGB):
                _finalize_bank(nc, sp, gbanks[k], qf, acc, k)
            accsum = sp.tile([128, 1], f32, tag="accsum")
            nc.vector.reduce_sum(accsum[:], acc[:], axis=mybir.AxisListType.X)
            tot = pg.tile([1, 1], f32, tag="tot")
            nc.tensor.matmul(tot[:], accsum[:], ones[:], start=True, stop=True)
            res = sp.tile([1, 1], f32, tag="res")
            nc.scalar.copy(res[:], tot[:])
            nc.sync.dma_start(out_d[:], res[:])

    nc.compile()
    return nc


def _get_program():
    global _prog
    if _prog is None:
        _prog = _build_program()
    return _prog


def _bf16_split(x32, parts):
    """Split fp32 array into `parts` bf16 arrays summing to x32 (greedy)."""
    out = []
    rem = x32.astype(np.float64)
    for _ in range(parts):
        p = rem.astype(np.float32).astype(BF16)
        out.append(p)
        rem = rem - p.astype(np.float64)
    return out


def _host_prep(q, r):
    q = np.ascontiguousarray(np.asarray(q, np.float32))
    r = np.ascontiguousarray(np.asarray(r, np.float32))
    r64 = r.astype(np.float64)
    s64 = (r64 * r64).sum(1)

    rh, rl = _bf16_split(r, 2)  # [N,3] bf16 each
    m2rh, m2rl = (-2.0 * rh.astype(np.float32)).astype(BF16), (
        -2.0 * rl.astype(np.float32)
    ).astype(BF16)
    sh, sm, sl = _bf16_split(s64, 3)  # [N] bf16 each
    onesN = np.ones(N, BF16)

    # rhs rows (i side) pair with lhsT rows (j side), K=18:
    #   -2rh_j*rh_i, -2rh_j*rl_i, -2rl_j*rh_i, -2rl_j*rl_i (12 rows),
    #   (sh+sm+sl)_j * 1 (3 rows), 1 * (sh+sm+sl)_i (3 rows)
    A18 = np.concatenate(
        [rh.T, rl.T, rh.T, rl.T, [onesN, onesN, onesN], [sh, sm, sl]]
    ).astype(BF16)  # [18, N]
    B18 = np.concatenate(
        [m2rh.T, m2rh.T, m2rl.T, m2rl.T, [sh, sm, sl], [onesN, onesN, onesN]]
    ).astype(BF16)  # [18, N]

    qT = np.ascontiguousarray(q.T)  # [NQ, N] f32

    # 36 super-tiles of the symmetric pair grid: 8 diagonal (w=1, core-local
    # tile 0, diag-masked) + 28 upper-triangle pairs (w=2), dealt round-robin;
    # cores with only 3 pairs get a zero-weight dummy tile.
    pairs = [(a, b) for a in range(8) for b in range(a + 1, 8)]
    assignments = [[(c, c, 1.0)] for c in range(NCORES)]
    for idx, (a, b) in enumerate(pairs):
        assignments[idx % NCORES].append((a, b, 2.0))
    for c in range(NCORES):
        while len(assignments[c]) < NT:
            assignments[c].append((c, c, 0.0))

    in_maps = []
    for c in range(NCORES):
        tiles = assignments[c]  # (j-chunk a, i-chunk b, weight)
        AT = np.empty((18, NT * CH), BF16)
        BT = np.empty((18, NT * CH), BF16)
        QW = np.empty((128, NT * 64), BF16)
        QF = np.zeros((128, NGB * CH), np.float32)
        for t, (a, b, w) in enumerate(tiles):
            k, m = divmod(t, 4)
            AT[:, t * CH : (t + 1) * CH] = A18[:, b * CH : (b + 1) * CH]
            BT[:, t * CH : (t + 1) * CH] = B18[:, a * CH : (a + 1) * CH]
            # Finalize reads quadrant rows 32m + [0..16): both the qh and ql
            # halves of G contract against the same fp32 qT chunk.
            QF[32 * m : 32 * m + NQ, k * CH : (k + 1) * CH] = qT[
                :, b * CH : (b + 1) * CH
            ]
            QF[32 * m + NQ : 32 * m + 2 * NQ, k * CH : (k + 1) * CH] = qT[
                :, b * CH : (b + 1) * CH
            ]
            wq = (w * q[a * CH : (a + 1) * CH, :]).astype(np.float32)  # [512, NQ]
            wqh, wql = _bf16_split(wq, 2)
            blk = np.concatenate([wqh, wql], axis=1)  # [512, 16]
            QW[:, t * 64 : (t + 1) * 64] = (
                blk.reshape(4, 128, 2 * NQ).transpose(1, 0, 2).reshape(128, 64)
            )
        in_maps.append({"AT": AT, "BT": BT, "QW": QW, "QF": QF})
    return in_maps


def _diag_constant():
    """F value the device computes on the (unmasked) pair-grid diagonal.

    d2 on the diagonal is |err| < 2e-4, and F(x) = erf(sqrt((x+B)/2)) /
    sqrt(x+B) is flat there (variation < 1e-4 relative), so every diagonal
    element lands on the same bf16 value: bf16 applied to the device's
    erf * quadratic-reciprocal product at d = sqrt(BIAS). The bf16 bucket
    is 0.4% wide -- vastly wider than the variation -- so this is exact."""
    from scipy.special import erf as _erf

    d0 = np.float32(np.sqrt(BIAS))
    e0 = np.float32(_erf(float(d0) * INV_SQRT2))
    nx = (~d0.reshape(1).view(np.int32)).view(np.float32)[0]
    t = np.float32(d0 * nx)
    rq = np.float32(
        (np.float32(RECIP_A) + t * (np.float32(RECIP_B) + np.float32(RECIP_C) * t))
        * nx
    )
    return float((e0 * rq).astype(BF16))


def kernel(q, r, cell):
    global LAST_RESULTS
    in_maps = _host_prep(q, r)
    nc = _get_program()
    res = run_bass_kernel_spmd(nc, in_maps, list(range(NCORES)), trace=TRACE)
    LAST_RESULTS = res
    S = sum(float(res.results[c]["OUT"][0, 0]) for c in range(NCORES))
    S -= _diag_constant() * float((q.astype(np.float64) ** 2).sum())
    val = S / TWOPI / 2.0 * NORM_FACTOR
    return np.array([val], np.float32)
